# revision 29
# baseline (speedup 1.0000x reference)
"""Causal self-attention on 8 NeuronCores (Bass/Tile, fp8 DoubleRow).

Sharding: tensor-parallel over heads x data-parallel over batch.
  core c -> batch b = c//4, heads 4g..4g+3 where g = c%4.

Precision plan (validated in numpy sim, max-rel ~3.5e-3 vs 2e-2 gate):
  - tokens 0..511 ("stripe 0", small softmax sums -> no error averaging):
    3-product compensated fp8-DR QKV, bf16 scores/p/v attention.
  - tokens 512..2047: single-product fp8-DR QKV (x_hi*W_hi), fp8-DR
    scores (d=64 as 2x32 DoubleRow slots), fp8 p via ACT exp or DVE
    Schraudolph bit-trick (affine -> int8 -> e4m3 bits), fp8-DR pv with
    p stationary / v-augmented moving (65 rows per 256-token kblock pair).
  - scales folded host-side: W_qk*64 (k also /sqrt(64)), W_v*32,
    exp(psum/4096), W_proj/32; biases folded to match.
  - pv-B orientation puts the softmax denominator per-partition: one
    reciprocal [128,4,1] + one broadcast-multiply per query block.
  - yT via PE transpose (bf16), output projection bf16, bf16 partials
    DMA'd out; host sums 4 partials per batch in fp32 + b_proj.
"""

import os
import sys

for _p in ("/opt/trn_rl_repo", "/opt/pypackages"):
    if os.path.isdir(_p) and _p not in sys.path:
        sys.path.append(_p)

import numpy as np
import ml_dtypes

import concourse.bass as bass
import concourse.tile as tile
import concourse.mybir as mybir
from concourse import bacc
from concourse.bass_utils import run_bass_kernel_spmd

B, T, C = 2, 2048, 1024
H = 16            # total heads
D = 64            # head dim
HPC = 4           # heads per core
CH = HPC * D      # 256 channels per core
N_CORES = 8
NT = 4            # 512-token stripes

f32 = mybir.dt.float32
bf16 = mybir.dt.bfloat16
f8 = mybir.dt.float8e4
i8 = mybir.dt.int8
NP8 = ml_dtypes.float8_e4m3
NPB = ml_dtypes.bfloat16
ts = bass.ts
ds = bass.ds
DR = mybir.MatmulPerfMode.DoubleRow
Exp = mybir.ActivationFunctionType.Exp
MULT = mybir.AluOpType.mult
ADD = mybir.AluOpType.add

LOG2E = 1.4426950408889634
SCHR_C = 55.54            # e4m3 Schraudolph constant (DVE rounds to nearest)
EXP_SCALE = 1.0 / 4096.0  # q,k carry 64x each
DVE_TENTHS = 3            # fraction (in tenths) of full exp pairs on DVE

_COMPILED = None


def _build():
    nc = bacc.Bacc("TRN2", target_bir_lowering=False, debug=False,
                   num_devices=N_CORES)

    # input blobs: fp8 hi-weights (needed first), fp8 rest, bf16, f32
    N8A = 4096 + 2048                                # wqk wv
    N8B = 4096 + 2048 + 128 + 256                    # wqkl wvl S8 S8z
    N16 = 2048 + 128 + 128                           # wpt S16 eye
    N32 = 2 + 2 + 256                                # bq2 bk2 bvb
    blob8a_d = nc.dram_tensor("blob8a", [128, N8A], f8,
                              kind="ExternalInput").ap()
    blob8b_d = nc.dram_tensor("blob8b", [128, N8B], f8,
                              kind="ExternalInput").ap()
    blob16_d = nc.dram_tensor("blob16", [128, N16], bf16,
                              kind="ExternalInput").ap()
    blob32_d = nc.dram_tensor("blob32", [128, N32], f32,
                              kind="ExternalInput").ap()
    xhi_d = nc.dram_tensor("xhi", [128, 4, 2, T], f8, kind="ExternalInput").ap()
    xlo_d = nc.dram_tensor("xlo", [128, 4, 2, 512], f8, kind="ExternalInput").ap()
    out_d = nc.dram_tensor("out_partial", [T, C], bf16, kind="ExternalOutput").ap()

    with tile.TileContext(nc) as tc:
        with tc.tile_pool(name="consts", bufs=1) as consts, \
             tc.tile_pool(name="act", bufs=1) as act, \
             tc.tile_pool(name="xp", bufs=2) as xp, \
             tc.tile_pool(name="pp", bufs=34) as pp, \
             tc.tile_pool(name="p16", bufs=5) as p16p, \
             tc.tile_pool(name="ysb", bufs=3) as ysbp, \
             tc.tile_pool(name="rp", bufs=4) as rp, \
             tc.tile_pool(name="op", bufs=4) as op, \
             tc.tile_pool(name="ps_v", bufs=2, space="PSUM") as ps_v, \
             tc.tile_pool(name="ps_s", bufs=2, space="PSUM") as ps_s, \
             tc.tile_pool(name="ps_y", bufs=2, space="PSUM") as ps_y:

            # ---- constants: staged blob DMAs (hi-weights + stripe-0 x
            #      first so qkv(0) starts ASAP) ----
            blob8a = consts.tile([128, N8A], f8)
            blob8b = consts.tile([128, N8B], f8)
            blob16 = consts.tile([128, N16], bf16)
            blob32 = consts.tile([128, N32], f32)
            xhi_sb = consts.tile([128, 4, 2, T], f8)
            xlo_sb = consts.tile([128, 4, 2, 512], f8)
            nc.sync.dma_start(blob8a[:], blob8a_d)
            nc.sync.dma_start(xhi_sb[:], xhi_d)
            nc.sync.dma_start(blob32[:], blob32_d)
            nc.sync.dma_start(blob8b[:], blob8b_d)
            nc.sync.dma_start(xlo_sb[:], xlo_d)
            nc.sync.dma_start(blob16[:], blob16_d)

            wqk_sb = blob8a[:, 0:4096].rearrange("p (c s f) -> p c s f",
                                                 c=4, s=2)
            wv_sb = blob8a[:, 4096:6144].rearrange("p (c s f) -> p c s f",
                                                   c=4, s=2)
            wqkl_sb = blob8b[:, 0:4096].rearrange("p (c s f) -> p c s f",
                                                  c=4, s=2)
            wvl_sb = blob8b[:, 4096:6144].rearrange("p (c s f) -> p c s f",
                                                    c=4, s=2)
            s8_sb = blob8b[:, 6144:6272]
            s8z_sb = blob8b[:, 6272:6528]
            wpt_sb = blob16[:, 0:2048].rearrange("p (s o) -> p s o", s=2)
            s16_sb = blob16[:, 2048:2176]
            eye_sb = blob16[:, 2176:2304]
            bq2_sb = blob32[:, 0:2].rearrange("p (s o) -> p s o", o=1)
            bk2_sb = blob32[:, 2:4].rearrange("p (s o) -> p s o", o=1)
            bvb_sb = blob32[:, 4:260].rearrange("p (o h d) -> p o h d",
                                                o=1, h=HPC)

            # ---- persistent activations ----
            # per head-pair tiles (SBUF AP base partition must be 0/32/64,
            # so head 3 cannot live at partitions 96..127 of a 128-tile)
            q8s = [act.tile([64, 2, T], f8, name=f"q8_{i}") for i in range(2)]
            k8s = [act.tile([64, 2, T], f8, name=f"k8_{i}") for i in range(2)]
            qb16s = [act.tile([64, 2, 512], bf16, name=f"qb_{i}")
                     for i in range(2)]
            kb16s = [act.tile([64, 2, 512], bf16, name=f"kb_{i}")
                     for i in range(2)]
            vaug8 = act.tile([128, 16, HPC, D + 1], f8)   # [tok,blk,h,d|1] 32v
            vaugb = act.tile([128, 4, HPC, D + 1], bf16)  # stripe-0 blocks
            yT = act.tile([128, 2, T], bf16)      # [ch-in-slab, slab, t] 32y

            nc.vector.memset(vaug8[:, :, :, D:D + 1], 1.0)
            nc.vector.memset(vaugb[:, :, :, D:D + 1], 1.0)

            def qkv_parts(ti):
                """QKV for stripe ti as 4 independently-emittable parts so
                they can interleave into the previous stripe's attention."""
                xhi = xhi_sb[:, :, :, ts(ti, 512)]
                prods = [(xhi, wqk_sb, wv_sb)]
                if ti == 0:
                    prods += [(xlo_sb, wqk_sb, wv_sb), (xhi, wqkl_sb, wvl_sb)]
                n = len(prods) * 4

                def qk_part(name, f0, bias, d8, d16):
                    def run():
                        ps = ps_s.tile([128, 2, 512], f32, tag="sc",
                                       name=f"ps_{name}{ti}")
                        for a in range(2):
                            j = 0
                            for xa, wa, _ in prods:
                                for ci in range(4):
                                    nc.tensor.matmul(
                                        ps[:, a, :],
                                        wa[:, ci, :, ds(f0 + 128 * a, 128)],
                                        xa[:, ci, :, :],
                                        start=(j == 0), stop=(j == n - 1),
                                        perf_mode=DR)
                                    j += 1
                        for hh in range(2):
                            psl = ps[ds(64 * hh, 64), :, :]
                            bl = bias[ds(64 * hh, 64), :, :].to_broadcast(
                                [64, 2, 512])
                            if ti == 0:
                                nc.vector.tensor_tensor(
                                    out=d16[hh][:], in0=psl, in1=bl, op=ADD)
                                if name == "k":
                                    nc.vector.tensor_tensor(
                                        out=d8[hh][:, :, 0:512], in0=psl,
                                        in1=bl, op=ADD)
                            else:
                                nc.vector.tensor_tensor(
                                    out=d8[hh][:, :, ts(ti, 512)], in0=psl,
                                    in1=bl, op=ADD)
                    return run

                def v_part(half):
                    def run():
                        pv = ps_v.tile([128, 2, 256], f32, tag="vps",
                                       name=f"vps{ti}_{half}")
                        for tj2 in range(2):
                            tj = 2 * half + tj2
                            j = 0
                            for xa, _, wva in prods:
                                for ci in range(4):
                                    nc.tensor.matmul(
                                        pv[:, tj2, :],
                                        xa[:, ci, :, ds(128 * tj, 128)],
                                        wva[:, ci, :, :],
                                        start=(j == 0), stop=(j == n - 1),
                                        perf_mode=DR)
                                    j += 1
                        blk = 4 * ti + 2 * half
                        pv4 = pv[:].rearrange("p b (h d) -> p b h d", h=HPC)
                        bvb4 = bvb_sb.to_broadcast([128, 2, HPC, D])
                        nc.vector.tensor_tensor(
                            out=vaug8[:, ds(blk, 2), :, 0:D],
                            in0=pv4, in1=bvb4, op=ADD)
                        if ti == 0:
                            nc.vector.tensor_tensor(
                                out=vaugb[:, ds(blk, 2), :, 0:D],
                                in0=pv4, in1=bvb4, op=ADD)
                    return run

                return [qk_part("q", 0, bq2_sb, q8s, qb16s),
                        qk_part("k", 256, bk2_sb, k8s, kb16s),
                        v_part(0), v_part(1)]

            def attn0(extra=()):
                extra = list(extra)
                for h in range(HPC):
                    if extra:
                        extra.pop(0)()
                    hp, hh = 32 * (h % 2), h // 2
                    py = ps_y.tile([D + 1, 512], f32, tag="y")
                    for m in range(2):
                        psc = ps_s.tile([128, 2, 512], f32, tag="sc")
                        for i in range(2):
                            ki = 2 * m + i
                            q0 = 128 * ki
                            for sl in range(2):
                                nc.tensor.matmul(
                                    psc[:, i, q0:],
                                    kb16s[hh][hp:hp + 32, sl, ts(ki, 128)],
                                    qb16s[hh][hp:hp + 32, sl, q0:],
                                    start=(sl == 0), stop=(sl == 1))
                            p = p16p.tile([128, 512], bf16, tag="p16")
                            nc.scalar.activation(p[:, q0:], psc[:, i, q0:],
                                                 Exp, scale=EXP_SCALE)
                            nc.gpsimd.tensor_tensor(
                                out=p[:, q0:q0 + 128], in0=p[:, q0:q0 + 128],
                                in1=s16_sb, op=MULT)
                            nc.tensor.matmul(
                                py[:, q0:], vaugb[:, ki, h, :], p[:, q0:],
                                start=(ki == 0), stop=(ki == 3))
                    rec = rp.tile([1, 512], f32, tag="rec")
                    nc.vector.reciprocal(rec[:], py[D:D + 1, :])
                    bc = rp.tile([D, 512], f32, tag="bc")
                    nc.gpsimd.partition_broadcast(bc[:], rec[:], channels=D)
                    nc.vector.tensor_tensor(
                        out=yT[ds(64 * (h % 2), D), h // 2, 0:512],
                        in0=py[0:D, :], in1=bc[:], op=MULT)

            def attn(qi, fuse_oproj=False, extra=()):
                extra = list(extra)
                npair = 2 * qi + 2

                def mmax(qb):
                    return (4 * qi + qb) // 2

                ph = [[None] * npair for _ in range(HPC)]
                pybs = {}
                ot = op.tile([128, 4, 1024], bf16, tag="ot", name="ot3") if fuse_oproj \
                    else None

                def scores_exp(h, m):
                    hp, hh = 32 * (h % 2), h // 2
                    diag = m >= 2 * qi
                    q0 = 256 if m == 2 * qi + 1 else 0
                    psc = ps_s.tile([128, 2, 512], f32, tag="sc")
                    for i in range(2):
                        ki = 2 * m + i
                        nc.tensor.matmul(
                            psc[:, i, q0:],
                            k8s[hh][hp:hp + 32, :, ts(ki, 128)],
                            q8s[hh][hp:hp + 32, :,
                                    ds(512 * qi + q0, 512 - q0)],
                            start=True, stop=True, perf_mode=DR)
                    p = pp.tile([128, 2, 512], f8, tag="p")
                    use_dve = (not diag) and ((h * 3 + m * 5 + qi * 7)
                                              % 10 < DVE_TENTHS)
                    if use_dve:
                        nc.vector.tensor_scalar(
                            out=p[:].bitcast(i8), in0=psc[:],
                            scalar1=8.0 * LOG2E * EXP_SCALE,
                            scalar2=SCHR_C, op0=MULT, op1=ADD)
                    else:
                        nc.scalar.activation(
                            p[:, :, q0:], psc[:, :, q0:],
                            Exp, scale=EXP_SCALE)
                    if diag:
                        nc.gpsimd.tensor_tensor(
                            out=p[:, 0, q0:q0 + 128],
                            in0=p[:, 0, q0:q0 + 128], in1=s8_sb, op=MULT)
                        nc.gpsimd.tensor_tensor(
                            out=p[:, 1, q0:q0 + 256],
                            in0=p[:, 1, q0:q0 + 256], in1=s8z_sb, op=MULT)
                    ph[h][m] = p

                def pvb_step(qb, h, m):
                    # one open accumulation group per bank at a time:
                    # h-major order keeps groups sequential
                    nc.tensor.matmul(
                        pybs[qb][:, h, :],
                        ph[h][m][:, :, ts(qb, 128)],
                        vaug8[:, ds(2 * m, 2), h, :],
                        start=(m == 0), stop=(m == mmax(qb)),
                        perf_mode=DR)

                def finish(qb):
                    pyb = pybs[qb]
                    rec = rp.tile([128, HPC, 1], f32, tag="recb")
                    nc.vector.reciprocal(rec[:], pyb[:, :, D:D + 1])
                    y_sb = ysbp.tile([128, HPC, D], bf16, tag="ysb")
                    nc.vector.tensor_tensor(
                        out=y_sb[:], in0=pyb[:, :, 0:D],
                        in1=rec[:].to_broadcast([128, HPC, D]), op=MULT)
                    tps = ps_y.tile([128, 2, 128], bf16, tag="y")
                    for s in range(2):
                        nc.tensor.transpose(
                            tps[:, s, :],
                            y_sb[:, ds(2 * s, 2), :].rearrange(
                                "p h d -> p (h d)"),
                            eye_sb)
                    nc.vector.tensor_copy(
                        yT[:, :, ds(512 * qi + 128 * qb, 128)], tps[:])
                    if fuse_oproj:
                        tg = 4 * qi + qb
                        for oi in range(2):
                            po = ps_s.tile([128, 512], f32, tag="sc",
                                           name=f"pof{tg}_{oi}")
                            for s in range(2):
                                nc.tensor.matmul(
                                    po[:], yT[:, s, ts(tg, 128)],
                                    wpt_sb[:, s, ts(oi, 512)],
                                    start=(s == 0), stop=(s == 1))
                            if (qb + oi) % 2 == 0:
                                nc.scalar.copy(ot[:, qb, ts(oi, 512)], po[:])
                            else:
                                nc.vector.tensor_copy(
                                    ot[:, qb, ts(oi, 512)], po[:])

                # qb 0,1 chains interleave with scores/exp at pair
                # granularity; h-major so each psum bank has exactly one
                # open accumulation group at any time
                pybs[0] = ps_y.tile([128, HPC, D + 1], f32, tag="y", name="pyb0")
                pybs[1] = ps_y.tile([128, HPC, D + 1], f32, tag="y", name="pyb1")
                for h in range(HPC):
                    if extra:
                        extra.pop(0)()
                    for m in range(npair):
                        scores_exp(h, m)
                        for qb in (0, 1):
                            if m <= mmax(qb):
                                pvb_step(qb, h, m)
                finish(0)
                finish(1)
                while extra:
                    extra.pop(0)()
                # qb 2,3 trail (overlap with next stripe's qkv/scores)
                pybs[2] = ps_y.tile([128, HPC, D + 1], f32, tag="y", name="pyb2")
                pybs[3] = ps_y.tile([128, HPC, D + 1], f32, tag="y", name="pyb3")
                for h in range(HPC):
                    for m in range(npair):
                        for qb in (2, 3):
                            if m <= mmax(qb):
                                pvb_step(qb, h, m)
                finish(2)
                finish(3)
                if fuse_oproj:
                    si = qi
                    nc.sync.dma_start(
                        out_d[ds(512 * si, 512), :].rearrange(
                            "(g p) c -> p g c", g=4), ot[:])

            def oproj_parts(si):
                ot = op.tile([128, 4, 1024], bf16, tag="ot", name=f"ot{si}")

                def tg_part(g):
                    def run():
                        tg = 4 * si + g
                        for oi in range(2):
                            po = ps_s.tile([128, 512], f32, tag="sc",
                                           name=f"po{tg}_{oi}")
                            for s in range(2):
                                nc.tensor.matmul(
                                    po[:], yT[:, s, ts(tg, 128)],
                                    wpt_sb[:, s, ts(oi, 512)],
                                    start=(s == 0), stop=(s == 1))
                            if (g + oi) % 2 == 0:
                                nc.scalar.copy(ot[:, g, ts(oi, 512)], po[:])
                            else:
                                nc.vector.tensor_copy(
                                    ot[:, g, ts(oi, 512)], po[:])
                        if g == 3:
                            nc.sync.dma_start(
                                out_d[ds(512 * si, 512), :].rearrange(
                                    "(g p) c -> p g c", g=4), ot[:])
                    return run
                return [tg_part(g) for g in range(4)]

            for part in qkv_parts(0):
                part()
            attn0(extra=qkv_parts(1))
            attn(1, extra=qkv_parts(2) + oproj_parts(0))
            attn(2, extra=qkv_parts(3) + oproj_parts(1))
            attn(3, fuse_oproj=True, extra=oproj_parts(2))

    nc.compile()
    return nc


def _get_compiled():
    global _COMPILED
    if _COMPILED is None:
        _COMPILED = _build()
    return _COMPILED


def _split8(a):
    hi = a.astype(NP8)
    lo = (a - hi.astype(np.float32)).astype(NP8)
    return hi, lo


def _host_prep(x, W_attn, b_attn, W_proj, b_proj):
    scale = np.float32(1.0 / np.sqrt(D))
    dd = np.arange(128)
    S8np = (np.arange(128)[None, :] >= dd[:, None])
    S8 = S8np.astype(NP8)
    S8z = np.concatenate(
        [np.zeros((128, 128), NP8), S8], axis=1)
    S16 = S8np.astype(NPB)
    eye = np.eye(128, dtype=NPB)

    # x in DR layout [p, ci, slot, t] per batch
    xhis, xlos = [], []
    for b in range(B):
        xt = np.ascontiguousarray(x[b].T)          # [C, T]
        xr = xt.reshape(4, 2, 128, T).transpose(2, 0, 1, 3)  # [p, ci, s, T]
        hi, lo = _split8(xr)
        xhis.append(np.ascontiguousarray(hi))
        xlos.append(np.ascontiguousarray(lo[:, :, :, :512]))

    in_maps = []
    for c in range(N_CORES):
        b, g = divmod(c, 4)
        ch0 = CH * g
        # output-channel orderings
        # q/k rows: f = 128*half + 32*h + dd  ->  qchan = 64*(4g+h) + 32*half + dd
        h_i = np.arange(128) // 32
        dd_i = np.arange(128) % 32
        qk_rows = np.concatenate(
            [ch0 + 64 * h_i + 32 * half + dd_i for half in range(2)])
        v_rows = ch0 + np.arange(256)              # 64*h + d order
        Wq = 64.0 * W_attn[qk_rows]                      # [256, C]
        Wk = 64.0 * scale * W_attn[C + qk_rows]
        Wv = 32.0 * W_attn[2 * C + v_rows]
        # stationary layout [p, ci, slot, f]
        def wlay(Wm):
            # Wm [F, C] -> [p, ci, s, F]
            r = Wm.T.reshape(4, 2, 128, Wm.shape[0]).transpose(2, 0, 1, 3)
            return np.ascontiguousarray(r)
        Wqk = np.concatenate([wlay(Wq), wlay(Wk)], axis=3)  # [p,ci,s,512]
        Wvl = wlay(Wv)                                       # [p,ci,s,256]
        wqk_hi, wqk_lo = _split8(Wqk)
        wv_hi, wv_lo = _split8(Wvl)

        bq2 = np.stack([64.0 * b_attn[ch0 + 64 * h_i + 32 * half + dd_i]
                        for half in range(2)], axis=1).astype(np.float32)
        bk2 = np.stack([64.0 * scale * b_attn[C + ch0 + 64 * h_i + 32 * half + dd_i]
                        for half in range(2)], axis=1).astype(np.float32)
        bvb = np.ascontiguousarray(np.broadcast_to(
            32.0 * b_attn[2 * C + v_rows][None, :], (128, 256))).astype(np.float32)
        wpt = np.ascontiguousarray(
            (W_proj[:, ch0:ch0 + CH] / 32.0).T.reshape(2, 128, 1024)
            .transpose(1, 0, 2)).astype(NPB)

        blob8a = np.concatenate(
            [wqk_hi.reshape(128, -1), wv_hi.reshape(128, -1)], axis=1)
        blob8b = np.concatenate(
            [wqk_lo.reshape(128, -1), wv_lo.reshape(128, -1),
             S8, S8z], axis=1)
        blob16 = np.concatenate(
            [wpt.reshape(128, -1), S16, eye], axis=1)
        blob32 = np.concatenate(
            [bq2, bk2, bvb], axis=1).astype(np.float32)
        in_maps.append({
            "xhi": xhis[b], "xlo": xlos[b],
            "blob8a": np.ascontiguousarray(blob8a),
            "blob8b": np.ascontiguousarray(blob8b),
            "blob16": np.ascontiguousarray(blob16),
            "blob32": np.ascontiguousarray(blob32),
        })
    return in_maps


def kernel(x, W_attn, b_attn, W_proj, b_proj):
    x = np.asarray(x, dtype=np.float32)
    W_attn = np.asarray(W_attn, dtype=np.float32)
    b_attn = np.asarray(b_attn, dtype=np.float32)
    W_proj = np.asarray(W_proj, dtype=np.float32)
    b_proj = np.asarray(b_proj, dtype=np.float32)

    nc = _get_compiled()
    in_maps = _host_prep(x, W_attn, b_attn, W_proj, b_proj)
    res = run_bass_kernel_spmd(nc, in_maps, core_ids=list(range(N_CORES)))

    out = np.empty((B, T, C), dtype=np.float32)
    for b in range(B):
        acc = res.results[4 * b]["out_partial"].astype(np.float32)
        for g in range(1, 4):
            acc += res.results[4 * b + g]["out_partial"].astype(np.float32)
        out[b] = acc + b_proj
    return out


# revision 30
# speedup vs baseline: 1.0033x; 1.0033x over previous
"""Causal self-attention on 8 NeuronCores (Bass/Tile, fp8 DoubleRow).

Sharding: tensor-parallel over heads x data-parallel over batch.
  core c -> batch b = c//4, heads 4g..4g+3 where g = c%4.

Precision plan (validated in numpy sim, max-rel ~3.5e-3 vs 2e-2 gate):
  - tokens 0..511 ("stripe 0", small softmax sums -> no error averaging):
    3-product compensated fp8-DR QKV, bf16 scores/p/v attention.
  - tokens 512..2047: single-product fp8-DR QKV (x_hi*W_hi), fp8-DR
    scores (d=64 as 2x32 DoubleRow slots), fp8 p via ACT exp or DVE
    Schraudolph bit-trick (affine -> int8 -> e4m3 bits), fp8-DR pv with
    p stationary / v-augmented moving (65 rows per 256-token kblock pair).
  - scales folded host-side: W_qk*64 (k also /sqrt(64)), W_v*32,
    exp(psum/4096), W_proj/32; biases folded to match.
  - pv-B orientation puts the softmax denominator per-partition: one
    reciprocal [128,4,1] + one broadcast-multiply per query block.
  - yT via PE transpose (bf16), output projection bf16, bf16 partials
    DMA'd out; host sums 4 partials per batch in fp32 + b_proj.
"""

import os
import sys

for _p in ("/opt/trn_rl_repo", "/opt/pypackages"):
    if os.path.isdir(_p) and _p not in sys.path:
        sys.path.append(_p)

import numpy as np
import ml_dtypes

import concourse.bass as bass
import concourse.tile as tile
import concourse.mybir as mybir
from concourse import bacc
from concourse.bass_utils import run_bass_kernel_spmd

B, T, C = 2, 2048, 1024
H = 16            # total heads
D = 64            # head dim
HPC = 4           # heads per core
CH = HPC * D      # 256 channels per core
N_CORES = 8
NT = 4            # 512-token stripes

f32 = mybir.dt.float32
bf16 = mybir.dt.bfloat16
f8 = mybir.dt.float8e4
i8 = mybir.dt.int8
NP8 = ml_dtypes.float8_e4m3
NPB = ml_dtypes.bfloat16
ts = bass.ts
ds = bass.ds
DR = mybir.MatmulPerfMode.DoubleRow
Exp = mybir.ActivationFunctionType.Exp
MULT = mybir.AluOpType.mult
ADD = mybir.AluOpType.add

LOG2E = 1.4426950408889634
SCHR_C = 55.54            # e4m3 Schraudolph constant (DVE rounds to nearest)
EXP_SCALE = 1.0 / 4096.0  # q,k carry 64x each
DVE_FRAC = 0.40           # tail fraction of each stripe's exps on DVE

_COMPILED = None


def _build():
    nc = bacc.Bacc("TRN2", target_bir_lowering=False, debug=False,
                   num_devices=N_CORES)

    # input blobs: fp8 hi-weights (needed first), fp8 rest, bf16, f32
    N8A = 4096 + 2048                                # wqk wv
    N8B = 4096 + 2048 + 128 + 256                    # wqkl wvl S8 S8z
    N16 = 2048 + 128 + 128                           # wpt S16 eye
    N32 = 2 + 2 + 256                                # bq2 bk2 bvb
    blob8a_d = nc.dram_tensor("blob8a", [128, N8A], f8,
                              kind="ExternalInput").ap()
    blob8b_d = nc.dram_tensor("blob8b", [128, N8B], f8,
                              kind="ExternalInput").ap()
    blob16_d = nc.dram_tensor("blob16", [128, N16], bf16,
                              kind="ExternalInput").ap()
    blob32_d = nc.dram_tensor("blob32", [128, N32], f32,
                              kind="ExternalInput").ap()
    xhi_d = nc.dram_tensor("xhi", [128, 4, 2, T], f8, kind="ExternalInput").ap()
    xlo_d = nc.dram_tensor("xlo", [128, 4, 2, 512], f8, kind="ExternalInput").ap()
    out_d = nc.dram_tensor("out_partial", [T, C], bf16, kind="ExternalOutput").ap()

    with tile.TileContext(nc) as tc:
        with tc.tile_pool(name="consts", bufs=1) as consts, \
             tc.tile_pool(name="act", bufs=1) as act, \
             tc.tile_pool(name="xp", bufs=2) as xp, \
             tc.tile_pool(name="pp", bufs=34) as pp, \
             tc.tile_pool(name="p16", bufs=5) as p16p, \
             tc.tile_pool(name="ysb", bufs=3) as ysbp, \
             tc.tile_pool(name="rp", bufs=4) as rp, \
             tc.tile_pool(name="op", bufs=4) as op, \
             tc.tile_pool(name="ps_v", bufs=2, space="PSUM") as ps_v, \
             tc.tile_pool(name="ps_s", bufs=2, space="PSUM") as ps_s, \
             tc.tile_pool(name="ps_y", bufs=2, space="PSUM") as ps_y:

            # ---- constants: staged blob DMAs (hi-weights + stripe-0 x
            #      first so qkv(0) starts ASAP) ----
            blob8a = consts.tile([128, N8A], f8)
            blob8b = consts.tile([128, N8B], f8)
            blob16 = consts.tile([128, N16], bf16)
            blob32 = consts.tile([128, N32], f32)
            xhi_sb = consts.tile([128, 4, 2, T], f8)
            xlo_sb = consts.tile([128, 4, 2, 512], f8)
            nc.sync.dma_start(blob8a[:], blob8a_d)
            nc.sync.dma_start(xhi_sb[:, :, :, 0:512], xhi_d[:, :, :, 0:512])
            nc.sync.dma_start(blob8b[:], blob8b_d)
            nc.sync.dma_start(xlo_sb[:], xlo_d)
            nc.sync.dma_start(blob32[:], blob32_d)
            nc.sync.dma_start(xhi_sb[:, :, :, 512:T], xhi_d[:, :, :, 512:T])
            nc.sync.dma_start(blob16[:], blob16_d)

            wqk_sb = blob8a[:, 0:4096].rearrange("p (c s f) -> p c s f",
                                                 c=4, s=2)
            wv_sb = blob8a[:, 4096:6144].rearrange("p (c s f) -> p c s f",
                                                   c=4, s=2)
            wqkl_sb = blob8b[:, 0:4096].rearrange("p (c s f) -> p c s f",
                                                  c=4, s=2)
            wvl_sb = blob8b[:, 4096:6144].rearrange("p (c s f) -> p c s f",
                                                    c=4, s=2)
            s8_sb = blob8b[:, 6144:6272]
            s8z_sb = blob8b[:, 6272:6528]
            wpt_sb = blob16[:, 0:2048].rearrange("p (s o) -> p s o", s=2)
            s16_sb = blob16[:, 2048:2176]
            eye_sb = blob16[:, 2176:2304]
            bq2_sb = blob32[:, 0:2].rearrange("p (s o) -> p s o", o=1)
            bk2_sb = blob32[:, 2:4].rearrange("p (s o) -> p s o", o=1)
            bvb_sb = blob32[:, 4:260].rearrange("p (o h d) -> p o h d",
                                                o=1, h=HPC)

            # ---- persistent activations ----
            # per head-pair tiles (SBUF AP base partition must be 0/32/64,
            # so head 3 cannot live at partitions 96..127 of a 128-tile)
            q8s = [act.tile([64, 2, T], f8, name=f"q8_{i}") for i in range(2)]
            k8s = [act.tile([64, 2, T], f8, name=f"k8_{i}") for i in range(2)]
            qb16s = [act.tile([64, 2, 512], bf16, name=f"qb_{i}")
                     for i in range(2)]
            kb16s = [act.tile([64, 2, 512], bf16, name=f"kb_{i}")
                     for i in range(2)]
            vaug8 = act.tile([128, 16, HPC, D + 1], f8)   # [tok,blk,h,d|1] 32v
            vaugb = act.tile([128, 4, HPC, D + 1], bf16)  # stripe-0 blocks
            yT = act.tile([128, 2, T], bf16)      # [ch-in-slab, slab, t] 32y

            nc.vector.memset(vaug8[:, :, :, D:D + 1], 1.0)
            nc.vector.memset(vaugb[:, :, :, D:D + 1], 1.0)

            def qkv_parts(ti):
                """QKV for stripe ti as 4 independently-emittable parts so
                they can interleave into the previous stripe's attention."""
                xhi = xhi_sb[:, :, :, ts(ti, 512)]
                prods = [(xhi, wqk_sb, wv_sb)]
                if ti == 0:
                    prods += [(xlo_sb, wqk_sb, wv_sb), (xhi, wqkl_sb, wvl_sb)]
                n = len(prods) * 4

                def qk_part(name, f0, bias, d8, d16):
                    def run():
                        ps = ps_s.tile([128, 2, 512], f32, tag="sc",
                                       name=f"ps_{name}{ti}")
                        for a in range(2):
                            j = 0
                            for xa, wa, _ in prods:
                                for ci in range(4):
                                    nc.tensor.matmul(
                                        ps[:, a, :],
                                        wa[:, ci, :, ds(f0 + 128 * a, 128)],
                                        xa[:, ci, :, :],
                                        start=(j == 0), stop=(j == n - 1),
                                        perf_mode=DR)
                                    j += 1
                        for hh in range(2):
                            psl = ps[ds(64 * hh, 64), :, :]
                            bl = bias[ds(64 * hh, 64), :, :].to_broadcast(
                                [64, 2, 512])
                            if ti == 0:
                                nc.vector.tensor_tensor(
                                    out=d16[hh][:], in0=psl, in1=bl, op=ADD)
                                if name == "k":
                                    nc.vector.tensor_tensor(
                                        out=d8[hh][:, :, 0:512], in0=psl,
                                        in1=bl, op=ADD)
                            else:
                                nc.vector.tensor_tensor(
                                    out=d8[hh][:, :, ts(ti, 512)], in0=psl,
                                    in1=bl, op=ADD)
                    return run

                def v_part(half):
                    def run():
                        pv = ps_v.tile([128, 2, 256], f32, tag="vps",
                                       name=f"vps{ti}_{half}")
                        for tj2 in range(2):
                            tj = 2 * half + tj2
                            j = 0
                            for xa, _, wva in prods:
                                for ci in range(4):
                                    nc.tensor.matmul(
                                        pv[:, tj2, :],
                                        xa[:, ci, :, ds(128 * tj, 128)],
                                        wva[:, ci, :, :],
                                        start=(j == 0), stop=(j == n - 1),
                                        perf_mode=DR)
                                    j += 1
                        blk = 4 * ti + 2 * half
                        pv4 = pv[:].rearrange("p b (h d) -> p b h d", h=HPC)
                        bvb4 = bvb_sb.to_broadcast([128, 2, HPC, D])
                        nc.vector.tensor_tensor(
                            out=vaug8[:, ds(blk, 2), :, 0:D],
                            in0=pv4, in1=bvb4, op=ADD)
                        if ti == 0:
                            nc.vector.tensor_tensor(
                                out=vaugb[:, ds(blk, 2), :, 0:D],
                                in0=pv4, in1=bvb4, op=ADD)
                    return run

                return [qk_part("q", 0, bq2_sb, q8s, qb16s),
                        qk_part("k", 256, bk2_sb, k8s, kb16s),
                        v_part(0), v_part(1)]

            def attn0(extra=()):
                extra = list(extra)
                for h in range(HPC):
                    if extra:
                        extra.pop(0)()
                    hp, hh = 32 * (h % 2), h // 2
                    py = ps_y.tile([D + 1, 512], f32, tag="y")
                    for m in range(2):
                        psc = ps_s.tile([128, 2, 512], f32, tag="sc")
                        for i in range(2):
                            ki = 2 * m + i
                            q0 = 128 * ki
                            for sl in range(2):
                                nc.tensor.matmul(
                                    psc[:, i, q0:],
                                    kb16s[hh][hp:hp + 32, sl, ts(ki, 128)],
                                    qb16s[hh][hp:hp + 32, sl, q0:],
                                    start=(sl == 0), stop=(sl == 1))
                            p = p16p.tile([128, 512], bf16, tag="p16")
                            nc.scalar.activation(p[:, q0:], psc[:, i, q0:],
                                                 Exp, scale=EXP_SCALE)
                            nc.gpsimd.tensor_tensor(
                                out=p[:, q0:q0 + 128], in0=p[:, q0:q0 + 128],
                                in1=s16_sb, op=MULT)
                            nc.tensor.matmul(
                                py[:, q0:], vaugb[:, ki, h, :], p[:, q0:],
                                start=(ki == 0), stop=(ki == 3))
                    rec = rp.tile([1, 512], f32, tag="rec")
                    nc.vector.reciprocal(rec[:], py[D:D + 1, :])
                    bc = rp.tile([D, 512], f32, tag="bc")
                    nc.gpsimd.partition_broadcast(bc[:], rec[:], channels=D)
                    nc.vector.tensor_tensor(
                        out=yT[ds(64 * (h % 2), D), h // 2, 0:512],
                        in0=py[0:D, :], in1=bc[:], op=MULT)

            def attn(qi, fuse_oproj=False, extra=()):
                extra = list(extra)
                npair = 2 * qi + 2

                def mmax(qb):
                    return (4 * qi + qb) // 2

                ph = [[None] * npair for _ in range(HPC)]
                pybs = {}
                ot = op.tile([128, 4, 1024], bf16, tag="ot", name="ot3") if fuse_oproj \
                    else None

                def scores_exp(h, m):
                    hp, hh = 32 * (h % 2), h // 2
                    diag = m >= 2 * qi
                    q0 = 256 if m == 2 * qi + 1 else 0
                    psc = ps_s.tile([128, 2, 512], f32, tag="sc")
                    for i in range(2):
                        ki = 2 * m + i
                        nc.tensor.matmul(
                            psc[:, i, q0:],
                            k8s[hh][hp:hp + 32, :, ts(ki, 128)],
                            q8s[hh][hp:hp + 32, :,
                                    ds(512 * qi + q0, 512 - q0)],
                            start=True, stop=True, perf_mode=DR)
                    p = pp.tile([128, 2, 512], f8, tag="p")
                    # tail units go to DVE so ACT and DVE drain together at
                    # stripe end (in-order engines; finish() barriers)
                    use_dve = ((h * npair + m) >= (1.0 - DVE_FRAC) * HPC
                               * npair) and q0 == 0
                    if use_dve:
                        nc.vector.tensor_scalar(
                            out=p[:].bitcast(i8), in0=psc[:],
                            scalar1=8.0 * LOG2E * EXP_SCALE,
                            scalar2=SCHR_C, op0=MULT, op1=ADD)
                    else:
                        nc.scalar.activation(
                            p[:, :, q0:], psc[:, :, q0:],
                            Exp, scale=EXP_SCALE)
                    if diag:
                        nc.gpsimd.tensor_tensor(
                            out=p[:, 0, q0:q0 + 128],
                            in0=p[:, 0, q0:q0 + 128], in1=s8_sb, op=MULT)
                        nc.gpsimd.tensor_tensor(
                            out=p[:, 1, q0:q0 + 256],
                            in0=p[:, 1, q0:q0 + 256], in1=s8z_sb, op=MULT)
                    ph[h][m] = p

                def pvb_step(qb, h, m):
                    # one open accumulation group per bank at a time:
                    # h-major order keeps groups sequential
                    nc.tensor.matmul(
                        pybs[qb][:, h, :],
                        ph[h][m][:, :, ts(qb, 128)],
                        vaug8[:, ds(2 * m, 2), h, :],
                        start=(m == 0), stop=(m == mmax(qb)),
                        perf_mode=DR)

                def finish(qb):
                    pyb = pybs[qb]
                    rec = rp.tile([128, HPC, 1], f32, tag="recb")
                    nc.vector.reciprocal(rec[:], pyb[:, :, D:D + 1])
                    y_sb = ysbp.tile([128, HPC, D], bf16, tag="ysb")
                    nc.vector.tensor_tensor(
                        out=y_sb[:], in0=pyb[:, :, 0:D],
                        in1=rec[:].to_broadcast([128, HPC, D]), op=MULT)
                    tps = ps_y.tile([128, 2, 128], bf16, tag="y")
                    for s in range(2):
                        nc.tensor.transpose(
                            tps[:, s, :],
                            y_sb[:, ds(2 * s, 2), :].rearrange(
                                "p h d -> p (h d)"),
                            eye_sb)
                    nc.vector.tensor_copy(
                        yT[:, :, ds(512 * qi + 128 * qb, 128)], tps[:])
                    if fuse_oproj:
                        tg = 4 * qi + qb
                        for oi in range(2):
                            po = ps_s.tile([128, 512], f32, tag="sc",
                                           name=f"pof{tg}_{oi}")
                            for s in range(2):
                                nc.tensor.matmul(
                                    po[:], yT[:, s, ts(tg, 128)],
                                    wpt_sb[:, s, ts(oi, 512)],
                                    start=(s == 0), stop=(s == 1))
                            if (qb + oi) % 2 == 0:
                                nc.scalar.copy(ot[:, qb, ts(oi, 512)], po[:])
                            else:
                                nc.vector.tensor_copy(
                                    ot[:, qb, ts(oi, 512)], po[:])

                # qb 0,1 chains interleave with scores/exp at pair
                # granularity; h-major so each psum bank has exactly one
                # open accumulation group at any time
                pybs[0] = ps_y.tile([128, HPC, D + 1], f32, tag="y", name="pyb0")
                pybs[1] = ps_y.tile([128, HPC, D + 1], f32, tag="y", name="pyb1")
                for h in range(HPC):
                    if extra:
                        extra.pop(0)()
                    for m in range(npair):
                        scores_exp(h, m)
                        for qb in (0, 1):
                            if m <= mmax(qb):
                                pvb_step(qb, h, m)
                finish(0)
                finish(1)
                while extra:
                    extra.pop(0)()
                # qb 2,3 trail (overlap with next stripe's qkv/scores)
                pybs[2] = ps_y.tile([128, HPC, D + 1], f32, tag="y", name="pyb2")
                pybs[3] = ps_y.tile([128, HPC, D + 1], f32, tag="y", name="pyb3")
                for h in range(HPC):
                    for m in range(npair):
                        for qb in (2, 3):
                            if m <= mmax(qb):
                                pvb_step(qb, h, m)
                finish(2)
                finish(3)
                if fuse_oproj:
                    si = qi
                    nc.sync.dma_start(
                        out_d[ds(512 * si, 512), :].rearrange(
                            "(g p) c -> p g c", g=4), ot[:])

            def oproj_parts(si):
                ot = op.tile([128, 4, 1024], bf16, tag="ot", name=f"ot{si}")

                def tg_part(g):
                    def run():
                        tg = 4 * si + g
                        for oi in range(2):
                            po = ps_s.tile([128, 512], f32, tag="sc",
                                           name=f"po{tg}_{oi}")
                            for s in range(2):
                                nc.tensor.matmul(
                                    po[:], yT[:, s, ts(tg, 128)],
                                    wpt_sb[:, s, ts(oi, 512)],
                                    start=(s == 0), stop=(s == 1))
                            if (g + oi) % 2 == 0:
                                nc.scalar.copy(ot[:, g, ts(oi, 512)], po[:])
                            else:
                                nc.vector.tensor_copy(
                                    ot[:, g, ts(oi, 512)], po[:])
                        if g == 3:
                            nc.sync.dma_start(
                                out_d[ds(512 * si, 512), :].rearrange(
                                    "(g p) c -> p g c", g=4), ot[:])
                    return run
                return [tg_part(g) for g in range(4)]

            for part in qkv_parts(0):
                part()
            attn0(extra=qkv_parts(1))
            attn(1, extra=qkv_parts(2) + oproj_parts(0))
            attn(2, extra=qkv_parts(3) + oproj_parts(1))
            attn(3, fuse_oproj=True, extra=oproj_parts(2))

    nc.compile()
    return nc


def _get_compiled():
    global _COMPILED
    if _COMPILED is None:
        _COMPILED = _build()
    return _COMPILED


def _split8(a):
    hi = a.astype(NP8)
    lo = (a - hi.astype(np.float32)).astype(NP8)
    return hi, lo


def _host_prep(x, W_attn, b_attn, W_proj, b_proj):
    scale = np.float32(1.0 / np.sqrt(D))
    dd = np.arange(128)
    S8np = (np.arange(128)[None, :] >= dd[:, None])
    S8 = S8np.astype(NP8)
    S8z = np.concatenate(
        [np.zeros((128, 128), NP8), S8], axis=1)
    S16 = S8np.astype(NPB)
    eye = np.eye(128, dtype=NPB)

    # x in DR layout [p, ci, slot, t] per batch
    xhis, xlos = [], []
    for b in range(B):
        xt = np.ascontiguousarray(x[b].T)          # [C, T]
        xr = xt.reshape(4, 2, 128, T).transpose(2, 0, 1, 3)  # [p, ci, s, T]
        hi, lo = _split8(xr)
        xhis.append(np.ascontiguousarray(hi))
        xlos.append(np.ascontiguousarray(lo[:, :, :, :512]))

    in_maps = []
    for c in range(N_CORES):
        b, g = divmod(c, 4)
        ch0 = CH * g
        # output-channel orderings
        # q/k rows: f = 128*half + 32*h + dd  ->  qchan = 64*(4g+h) + 32*half + dd
        h_i = np.arange(128) // 32
        dd_i = np.arange(128) % 32
        qk_rows = np.concatenate(
            [ch0 + 64 * h_i + 32 * half + dd_i for half in range(2)])
        v_rows = ch0 + np.arange(256)              # 64*h + d order
        Wq = 64.0 * W_attn[qk_rows]                      # [256, C]
        Wk = 64.0 * scale * W_attn[C + qk_rows]
        Wv = 32.0 * W_attn[2 * C + v_rows]
        # stationary layout [p, ci, slot, f]
        def wlay(Wm):
            # Wm [F, C] -> [p, ci, s, F]
            r = Wm.T.reshape(4, 2, 128, Wm.shape[0]).transpose(2, 0, 1, 3)
            return np.ascontiguousarray(r)
        Wqk = np.concatenate([wlay(Wq), wlay(Wk)], axis=3)  # [p,ci,s,512]
        Wvl = wlay(Wv)                                       # [p,ci,s,256]
        wqk_hi, wqk_lo = _split8(Wqk)
        wv_hi, wv_lo = _split8(Wvl)

        bq2 = np.stack([64.0 * b_attn[ch0 + 64 * h_i + 32 * half + dd_i]
                        for half in range(2)], axis=1).astype(np.float32)
        bk2 = np.stack([64.0 * scale * b_attn[C + ch0 + 64 * h_i + 32 * half + dd_i]
                        for half in range(2)], axis=1).astype(np.float32)
        bvb = np.ascontiguousarray(np.broadcast_to(
            32.0 * b_attn[2 * C + v_rows][None, :], (128, 256))).astype(np.float32)
        wpt = np.ascontiguousarray(
            (W_proj[:, ch0:ch0 + CH] / 32.0).T.reshape(2, 128, 1024)
            .transpose(1, 0, 2)).astype(NPB)

        blob8a = np.concatenate(
            [wqk_hi.reshape(128, -1), wv_hi.reshape(128, -1)], axis=1)
        blob8b = np.concatenate(
            [wqk_lo.reshape(128, -1), wv_lo.reshape(128, -1),
             S8, S8z], axis=1)
        blob16 = np.concatenate(
            [wpt.reshape(128, -1), S16, eye], axis=1)
        blob32 = np.concatenate(
            [bq2, bk2, bvb], axis=1).astype(np.float32)
        in_maps.append({
            "xhi": xhis[b], "xlo": xlos[b],
            "blob8a": np.ascontiguousarray(blob8a),
            "blob8b": np.ascontiguousarray(blob8b),
            "blob16": np.ascontiguousarray(blob16),
            "blob32": np.ascontiguousarray(blob32),
        })
    return in_maps


def kernel(x, W_attn, b_attn, W_proj, b_proj):
    x = np.asarray(x, dtype=np.float32)
    W_attn = np.asarray(W_attn, dtype=np.float32)
    b_attn = np.asarray(b_attn, dtype=np.float32)
    W_proj = np.asarray(W_proj, dtype=np.float32)
    b_proj = np.asarray(b_proj, dtype=np.float32)

    nc = _get_compiled()
    in_maps = _host_prep(x, W_attn, b_attn, W_proj, b_proj)
    res = run_bass_kernel_spmd(nc, in_maps, core_ids=list(range(N_CORES)))

    out = np.empty((B, T, C), dtype=np.float32)
    for b in range(B):
        acc = res.results[4 * b]["out_partial"].astype(np.float32)
        for g in range(1, 4):
            acc += res.results[4 * b + g]["out_partial"].astype(np.float32)
        out[b] = acc + b_proj
    return out


# revision 31
# speedup vs baseline: 1.0056x; 1.0023x over previous
"""Causal self-attention on 8 NeuronCores (Bass/Tile, fp8 DoubleRow).

Sharding: tensor-parallel over heads x data-parallel over batch.
  core c -> batch b = c//4, heads 4g..4g+3 where g = c%4.

Precision plan (validated in numpy sim, max-rel ~3.5e-3 vs 2e-2 gate):
  - tokens 0..511 ("stripe 0", small softmax sums -> no error averaging):
    3-product compensated fp8-DR QKV, bf16 scores/p/v attention.
  - tokens 512..2047: single-product fp8-DR QKV (x_hi*W_hi), fp8-DR
    scores (d=64 as 2x32 DoubleRow slots), fp8 p via ACT exp or DVE
    Schraudolph bit-trick (affine -> int8 -> e4m3 bits), fp8-DR pv with
    p stationary / v-augmented moving (65 rows per 256-token kblock pair).
  - scales folded host-side: W_qk*64 (k also /sqrt(64)), W_v*32,
    exp(psum/4096), W_proj/32; biases folded to match.
  - pv-B orientation puts the softmax denominator per-partition: one
    reciprocal [128,4,1] + one broadcast-multiply per query block.
  - yT via PE transpose (bf16), output projection bf16, bf16 partials
    DMA'd out; host sums 4 partials per batch in fp32 + b_proj.
"""

import os
import sys

for _p in ("/opt/trn_rl_repo", "/opt/pypackages"):
    if os.path.isdir(_p) and _p not in sys.path:
        sys.path.append(_p)

import numpy as np
import ml_dtypes

import concourse.bass as bass
import concourse.tile as tile
import concourse.mybir as mybir
from concourse import bacc
from concourse.bass_utils import run_bass_kernel_spmd

B, T, C = 2, 2048, 1024
H = 16            # total heads
D = 64            # head dim
HPC = 4           # heads per core
CH = HPC * D      # 256 channels per core
N_CORES = 8
NT = 4            # 512-token stripes

f32 = mybir.dt.float32
bf16 = mybir.dt.bfloat16
f8 = mybir.dt.float8e4
i8 = mybir.dt.int8
NP8 = ml_dtypes.float8_e4m3
NPB = ml_dtypes.bfloat16
ts = bass.ts
ds = bass.ds
DR = mybir.MatmulPerfMode.DoubleRow
Exp = mybir.ActivationFunctionType.Exp
MULT = mybir.AluOpType.mult
ADD = mybir.AluOpType.add

LOG2E = 1.4426950408889634
SCHR_C = 55.54            # e4m3 Schraudolph constant (DVE rounds to nearest)
EXP_SCALE = 1.0 / 4096.0  # q,k carry 64x each
DVE_FRAC = 0.40           # tail fraction of each stripe's exps on DVE

_COMPILED = None


def _build():
    nc = bacc.Bacc("TRN2", target_bir_lowering=False, debug=False,
                   num_devices=N_CORES)

    # input blobs: fp8 hi-weights (needed first), fp8 rest, bf16, f32
    N8A = 4096 + 2048                                # wqk wv
    N8B = 4096 + 2048 + 128 + 256                    # wqkl wvl S8 S8z
    N16 = 2048 + 128 + 128                           # wpt S16 eye
    N32 = 2 + 2 + 256                                # bq2 bk2 bvb
    blob8a_d = nc.dram_tensor("blob8a", [128, N8A], f8,
                              kind="ExternalInput").ap()
    blob8b_d = nc.dram_tensor("blob8b", [128, N8B], f8,
                              kind="ExternalInput").ap()
    blob16_d = nc.dram_tensor("blob16", [128, N16], bf16,
                              kind="ExternalInput").ap()
    blob32_d = nc.dram_tensor("blob32", [128, N32], f32,
                              kind="ExternalInput").ap()
    xhi_d = nc.dram_tensor("xhi", [128, 4, 2, T], f8, kind="ExternalInput").ap()
    xlo_d = nc.dram_tensor("xlo", [128, 4, 2, 512], f8, kind="ExternalInput").ap()
    out_d = nc.dram_tensor("out_partial", [T, C], bf16, kind="ExternalOutput").ap()

    with tile.TileContext(nc) as tc:
        with tc.tile_pool(name="consts", bufs=1) as consts, \
             tc.tile_pool(name="act", bufs=1) as act, \
             tc.tile_pool(name="xp", bufs=2) as xp, \
             tc.tile_pool(name="pp", bufs=34) as pp, \
             tc.tile_pool(name="p16", bufs=5) as p16p, \
             tc.tile_pool(name="ysb", bufs=3) as ysbp, \
             tc.tile_pool(name="rp", bufs=4) as rp, \
             tc.tile_pool(name="op", bufs=4) as op, \
             tc.tile_pool(name="ps_v", bufs=2, space="PSUM") as ps_v, \
             tc.tile_pool(name="ps_s", bufs=2, space="PSUM") as ps_s, \
             tc.tile_pool(name="ps_y", bufs=2, space="PSUM") as ps_y:

            # ---- constants: staged blob DMAs (hi-weights + stripe-0 x
            #      first so qkv(0) starts ASAP) ----
            blob8a = consts.tile([128, N8A], f8)
            blob8b = consts.tile([128, N8B], f8)
            blob16 = consts.tile([128, N16], bf16)
            blob32 = consts.tile([128, N32], f32)
            xhi_sb = consts.tile([128, 4, 2, T], f8)
            xlo_sb = consts.tile([128, 4, 2, 512], f8)
            nc.sync.dma_start(blob8a[:], blob8a_d)
            nc.sync.dma_start(xhi_sb[:, :, :, 0:512], xhi_d[:, :, :, 0:512])
            nc.sync.dma_start(blob8b[:], blob8b_d)
            nc.sync.dma_start(xlo_sb[:], xlo_d)
            nc.sync.dma_start(blob32[:], blob32_d)
            nc.sync.dma_start(xhi_sb[:, :, :, 512:T], xhi_d[:, :, :, 512:T])
            nc.sync.dma_start(blob16[:], blob16_d)

            wqk_sb = blob8a[:, 0:4096].rearrange("p (c s f) -> p c s f",
                                                 c=4, s=2)
            wv_sb = blob8a[:, 4096:6144].rearrange("p (c s f) -> p c s f",
                                                   c=4, s=2)
            wqkl_sb = blob8b[:, 0:4096].rearrange("p (c s f) -> p c s f",
                                                  c=4, s=2)
            wvl_sb = blob8b[:, 4096:6144].rearrange("p (c s f) -> p c s f",
                                                    c=4, s=2)
            s8_sb = blob8b[:, 6144:6272]
            s8z_sb = blob8b[:, 6272:6528]
            wpt_sb = blob16[:, 0:2048].rearrange("p (s o) -> p s o", s=2)
            s16_sb = blob16[:, 2048:2176]
            eye_sb = blob16[:, 2176:2304]
            bq2_sb = blob32[:, 0:2].rearrange("p (s o) -> p s o", o=1)
            bk2_sb = blob32[:, 2:4].rearrange("p (s o) -> p s o", o=1)
            bvb_sb = blob32[:, 4:260].rearrange("p (o h d) -> p o h d",
                                                o=1, h=HPC)

            # ---- persistent activations ----
            # per head-pair tiles (SBUF AP base partition must be 0/32/64,
            # so head 3 cannot live at partitions 96..127 of a 128-tile)
            q8s = [act.tile([64, 2, T], f8, name=f"q8_{i}") for i in range(2)]
            k8s = [act.tile([64, 2, T], f8, name=f"k8_{i}") for i in range(2)]
            qb16s = [act.tile([64, 2, 512], bf16, name=f"qb_{i}")
                     for i in range(2)]
            kb16s = [act.tile([64, 2, 512], bf16, name=f"kb_{i}")
                     for i in range(2)]
            vaug8 = act.tile([128, 16, HPC, D + 1], f8)   # [tok,blk,h,d|1] 32v
            vaugb = act.tile([128, 4, HPC, D + 1], bf16)  # stripe-0 blocks
            yT = act.tile([128, 2, T], bf16)      # [ch-in-slab, slab, t] 32y

            nc.vector.memset(vaug8[:, :, :, D:D + 1], 1.0)
            nc.vector.memset(vaugb[:, :, :, D:D + 1], 1.0)

            def qkv_parts(ti):
                """QKV for stripe ti as 4 independently-emittable parts so
                they can interleave into the previous stripe's attention."""
                xhi = xhi_sb[:, :, :, ts(ti, 512)]
                prods = [(xhi, wqk_sb, wv_sb)]
                if ti == 0:
                    prods += [(xlo_sb, wqk_sb, wv_sb), (xhi, wqkl_sb, wvl_sb)]
                n = len(prods) * 4

                def qk_part(name, f0, bias, d8, d16):
                    def run():
                        ps = ps_s.tile([128, 2, 512], f32, tag="sc",
                                       name=f"ps_{name}{ti}")
                        for a in range(2):
                            j = 0
                            for xa, wa, _ in prods:
                                for ci in range(4):
                                    nc.tensor.matmul(
                                        ps[:, a, :],
                                        wa[:, ci, :, ds(f0 + 128 * a, 128)],
                                        xa[:, ci, :, :],
                                        start=(j == 0), stop=(j == n - 1),
                                        perf_mode=DR)
                                    j += 1
                        for hh in range(2):
                            psl = ps[ds(64 * hh, 64), :, :]
                            bl = bias[ds(64 * hh, 64), :, :].to_broadcast(
                                [64, 2, 512])
                            if ti == 0:
                                nc.vector.tensor_tensor(
                                    out=d16[hh][:], in0=psl, in1=bl, op=ADD)
                                if name == "k":
                                    nc.vector.tensor_tensor(
                                        out=d8[hh][:, :, 0:512], in0=psl,
                                        in1=bl, op=ADD)
                            else:
                                nc.vector.tensor_tensor(
                                    out=d8[hh][:, :, ts(ti, 512)], in0=psl,
                                    in1=bl, op=ADD)
                    return run

                def v_part(half):
                    def run():
                        pv = ps_v.tile([128, 2, 256], f32, tag="vps",
                                       name=f"vps{ti}_{half}")
                        for tj2 in range(2):
                            tj = 2 * half + tj2
                            j = 0
                            for xa, _, wva in prods:
                                for ci in range(4):
                                    nc.tensor.matmul(
                                        pv[:, tj2, :],
                                        xa[:, ci, :, ds(128 * tj, 128)],
                                        wva[:, ci, :, :],
                                        start=(j == 0), stop=(j == n - 1),
                                        perf_mode=DR)
                                    j += 1
                        blk = 4 * ti + 2 * half
                        pv4 = pv[:].rearrange("p b (h d) -> p b h d", h=HPC)
                        bvb4 = bvb_sb.to_broadcast([128, 2, HPC, D])
                        nc.vector.tensor_tensor(
                            out=vaug8[:, ds(blk, 2), :, 0:D],
                            in0=pv4, in1=bvb4, op=ADD)
                        if ti == 0:
                            nc.vector.tensor_tensor(
                                out=vaugb[:, ds(blk, 2), :, 0:D],
                                in0=pv4, in1=bvb4, op=ADD)
                    return run

                return [qk_part("q", 0, bq2_sb, q8s, qb16s),
                        qk_part("k", 256, bk2_sb, k8s, kb16s),
                        v_part(0), v_part(1)]

            def attn0(extra=()):
                extra = list(extra)
                for h in range(HPC):
                    if extra:
                        extra.pop(0)()
                    hp, hh = 32 * (h % 2), h // 2
                    py = ps_y.tile([D + 1, 512], f32, tag="y")
                    pk = [None] * 4
                    psc = None
                    for ki in range(5):
                        if ki < 4:
                            q0 = 128 * ki
                            if ki % 2 == 0:
                                psc = ps_s.tile([128, 2, 512], f32, tag="sc",
                                                name=f"psc0_{h}_{ki}")
                            for sl in range(2):
                                nc.tensor.matmul(
                                    psc[:, ki % 2, q0:],
                                    kb16s[hh][hp:hp + 32, sl, ts(ki, 128)],
                                    qb16s[hh][hp:hp + 32, sl, q0:],
                                    start=(sl == 0), stop=(sl == 1))
                            p = p16p.tile([128, 512], bf16, tag="p16",
                                          name=f"p16_{h}_{ki}")
                            nc.scalar.activation(p[:, q0:], psc[:, ki % 2, q0:],
                                                 Exp, scale=EXP_SCALE)
                            nc.gpsimd.tensor_tensor(
                                out=p[:, q0:q0 + 128], in0=p[:, q0:q0 + 128],
                                in1=s16_sb, op=MULT)
                            pk[ki] = p
                        if ki >= 1:
                            kj = ki - 1
                            qj = 128 * kj
                            nc.tensor.matmul(
                                py[:, qj:], vaugb[:, kj, h, :], pk[kj][:, qj:],
                                start=(kj == 0), stop=(kj == 3))
                    rec = rp.tile([1, 512], f32, tag="rec")
                    nc.vector.reciprocal(rec[:], py[D:D + 1, :])
                    bc = rp.tile([D, 512], f32, tag="bc")
                    nc.gpsimd.partition_broadcast(bc[:], rec[:], channels=D)
                    nc.vector.tensor_tensor(
                        out=yT[ds(64 * (h % 2), D), h // 2, 0:512],
                        in0=py[0:D, :], in1=bc[:], op=MULT)

            def attn(qi, fuse_oproj=False, extra=()):
                extra = list(extra)
                npair = 2 * qi + 2

                def mmax(qb):
                    return (4 * qi + qb) // 2

                ph = [[None] * npair for _ in range(HPC)]
                pybs = {}
                ot = op.tile([128, 4, 1024], bf16, tag="ot", name="ot3") if fuse_oproj \
                    else None

                def scores_exp(h, m):
                    hp, hh = 32 * (h % 2), h // 2
                    diag = m >= 2 * qi
                    q0 = 256 if m == 2 * qi + 1 else 0
                    psc = ps_s.tile([128, 2, 512], f32, tag="sc")
                    for i in range(2):
                        ki = 2 * m + i
                        nc.tensor.matmul(
                            psc[:, i, q0:],
                            k8s[hh][hp:hp + 32, :, ts(ki, 128)],
                            q8s[hh][hp:hp + 32, :,
                                    ds(512 * qi + q0, 512 - q0)],
                            start=True, stop=True, perf_mode=DR)
                    p = pp.tile([128, 2, 512], f8, tag="p")
                    # tail units go to DVE so ACT and DVE drain together at
                    # stripe end (in-order engines; finish() barriers)
                    use_dve = ((h * npair + m) >= (1.0 - DVE_FRAC) * HPC
                               * npair) and q0 == 0
                    if use_dve:
                        nc.vector.tensor_scalar(
                            out=p[:].bitcast(i8), in0=psc[:],
                            scalar1=8.0 * LOG2E * EXP_SCALE,
                            scalar2=SCHR_C, op0=MULT, op1=ADD)
                    else:
                        nc.scalar.activation(
                            p[:, :, q0:], psc[:, :, q0:],
                            Exp, scale=EXP_SCALE)
                    if diag:
                        nc.gpsimd.tensor_tensor(
                            out=p[:, 0, q0:q0 + 128],
                            in0=p[:, 0, q0:q0 + 128], in1=s8_sb, op=MULT)
                        nc.gpsimd.tensor_tensor(
                            out=p[:, 1, q0:q0 + 256],
                            in0=p[:, 1, q0:q0 + 256], in1=s8z_sb, op=MULT)
                    ph[h][m] = p

                def pvb_step(qb, h, m):
                    # one open accumulation group per bank at a time:
                    # h-major order keeps groups sequential
                    nc.tensor.matmul(
                        pybs[qb][:, h, :],
                        ph[h][m][:, :, ts(qb, 128)],
                        vaug8[:, ds(2 * m, 2), h, :],
                        start=(m == 0), stop=(m == mmax(qb)),
                        perf_mode=DR)

                def finish(qb):
                    pyb = pybs[qb]
                    rec = rp.tile([128, HPC, 1], f32, tag="recb")
                    nc.vector.reciprocal(rec[:], pyb[:, :, D:D + 1])
                    y_sb = ysbp.tile([128, HPC, D], bf16, tag="ysb")
                    nc.vector.tensor_tensor(
                        out=y_sb[:], in0=pyb[:, :, 0:D],
                        in1=rec[:].to_broadcast([128, HPC, D]), op=MULT)
                    tps = ps_y.tile([128, 2, 128], bf16, tag="y")
                    for s in range(2):
                        nc.tensor.transpose(
                            tps[:, s, :],
                            y_sb[:, ds(2 * s, 2), :].rearrange(
                                "p h d -> p (h d)"),
                            eye_sb)
                    nc.vector.tensor_copy(
                        yT[:, :, ds(512 * qi + 128 * qb, 128)], tps[:])
                    if fuse_oproj:
                        tg = 4 * qi + qb
                        for oi in range(2):
                            po = ps_s.tile([128, 512], f32, tag="sc",
                                           name=f"pof{tg}_{oi}")
                            for s in range(2):
                                nc.tensor.matmul(
                                    po[:], yT[:, s, ts(tg, 128)],
                                    wpt_sb[:, s, ts(oi, 512)],
                                    start=(s == 0), stop=(s == 1))
                            if (qb + oi) % 2 == 0:
                                nc.scalar.copy(ot[:, qb, ts(oi, 512)], po[:])
                            else:
                                nc.vector.tensor_copy(
                                    ot[:, qb, ts(oi, 512)], po[:])

                # qb 0,1 chains interleave with scores/exp at pair
                # granularity; h-major so each psum bank has exactly one
                # open accumulation group at any time
                pybs[0] = ps_y.tile([128, HPC, D + 1], f32, tag="y", name="pyb0")
                pybs[1] = ps_y.tile([128, HPC, D + 1], f32, tag="y", name="pyb1")
                LAG = 2   # pv consumes exp output LAG pairs behind the
                          # scores so in-order PE never head-of-line blocks
                for h in range(HPC):
                    if extra:
                        extra.pop(0)()
                    for m in range(npair + LAG):
                        if m < npair:
                            scores_exp(h, m)
                        mm = m - LAG
                        if mm >= 0:
                            for qb in (0, 1):
                                if mm <= mmax(qb):
                                    pvb_step(qb, h, mm)
                finish(0)
                finish(1)
                while extra:
                    extra.pop(0)()
                # qb 2,3 trail (overlap with next stripe's qkv/scores)
                pybs[2] = ps_y.tile([128, HPC, D + 1], f32, tag="y", name="pyb2")
                pybs[3] = ps_y.tile([128, HPC, D + 1], f32, tag="y", name="pyb3")
                for h in range(HPC):
                    for m in range(npair):
                        for qb in (2, 3):
                            if m <= mmax(qb):
                                pvb_step(qb, h, m)
                finish(2)
                finish(3)
                if fuse_oproj:
                    si = qi
                    nc.sync.dma_start(
                        out_d[ds(512 * si, 512), :].rearrange(
                            "(g p) c -> p g c", g=4), ot[:])

            def oproj_parts(si):
                ot = op.tile([128, 4, 1024], bf16, tag="ot", name=f"ot{si}")

                def tg_part(g):
                    def run():
                        tg = 4 * si + g
                        for oi in range(2):
                            po = ps_s.tile([128, 512], f32, tag="sc",
                                           name=f"po{tg}_{oi}")
                            for s in range(2):
                                nc.tensor.matmul(
                                    po[:], yT[:, s, ts(tg, 128)],
                                    wpt_sb[:, s, ts(oi, 512)],
                                    start=(s == 0), stop=(s == 1))
                            if (g + oi) % 2 == 0:
                                nc.scalar.copy(ot[:, g, ts(oi, 512)], po[:])
                            else:
                                nc.vector.tensor_copy(
                                    ot[:, g, ts(oi, 512)], po[:])
                        if g == 3:
                            nc.sync.dma_start(
                                out_d[ds(512 * si, 512), :].rearrange(
                                    "(g p) c -> p g c", g=4), ot[:])
                    return run
                return [tg_part(g) for g in range(4)]

            for part in qkv_parts(0):
                part()
            attn0(extra=qkv_parts(1))
            attn(1, extra=qkv_parts(2) + oproj_parts(0))
            attn(2, extra=qkv_parts(3) + oproj_parts(1))
            attn(3, fuse_oproj=True, extra=oproj_parts(2))

    nc.compile()
    return nc


def _get_compiled():
    global _COMPILED
    if _COMPILED is None:
        _COMPILED = _build()
    return _COMPILED


def _split8(a):
    hi = a.astype(NP8)
    lo = (a - hi.astype(np.float32)).astype(NP8)
    return hi, lo


def _host_prep(x, W_attn, b_attn, W_proj, b_proj):
    scale = np.float32(1.0 / np.sqrt(D))
    dd = np.arange(128)
    S8np = (np.arange(128)[None, :] >= dd[:, None])
    S8 = S8np.astype(NP8)
    S8z = np.concatenate(
        [np.zeros((128, 128), NP8), S8], axis=1)
    S16 = S8np.astype(NPB)
    eye = np.eye(128, dtype=NPB)

    # x in DR layout [p, ci, slot, t] per batch
    xhis, xlos = [], []
    for b in range(B):
        xt = np.ascontiguousarray(x[b].T)          # [C, T]
        xr = xt.reshape(4, 2, 128, T).transpose(2, 0, 1, 3)  # [p, ci, s, T]
        hi, lo = _split8(xr)
        xhis.append(np.ascontiguousarray(hi))
        xlos.append(np.ascontiguousarray(lo[:, :, :, :512]))

    in_maps = []
    for c in range(N_CORES):
        b, g = divmod(c, 4)
        ch0 = CH * g
        # output-channel orderings
        # q/k rows: f = 128*half + 32*h + dd  ->  qchan = 64*(4g+h) + 32*half + dd
        h_i = np.arange(128) // 32
        dd_i = np.arange(128) % 32
        qk_rows = np.concatenate(
            [ch0 + 64 * h_i + 32 * half + dd_i for half in range(2)])
        v_rows = ch0 + np.arange(256)              # 64*h + d order
        Wq = 64.0 * W_attn[qk_rows]                      # [256, C]
        Wk = 64.0 * scale * W_attn[C + qk_rows]
        Wv = 32.0 * W_attn[2 * C + v_rows]
        # stationary layout [p, ci, slot, f]
        def wlay(Wm):
            # Wm [F, C] -> [p, ci, s, F]
            r = Wm.T.reshape(4, 2, 128, Wm.shape[0]).transpose(2, 0, 1, 3)
            return np.ascontiguousarray(r)
        Wqk = np.concatenate([wlay(Wq), wlay(Wk)], axis=3)  # [p,ci,s,512]
        Wvl = wlay(Wv)                                       # [p,ci,s,256]
        wqk_hi, wqk_lo = _split8(Wqk)
        wv_hi, wv_lo = _split8(Wvl)

        bq2 = np.stack([64.0 * b_attn[ch0 + 64 * h_i + 32 * half + dd_i]
                        for half in range(2)], axis=1).astype(np.float32)
        bk2 = np.stack([64.0 * scale * b_attn[C + ch0 + 64 * h_i + 32 * half + dd_i]
                        for half in range(2)], axis=1).astype(np.float32)
        bvb = np.ascontiguousarray(np.broadcast_to(
            32.0 * b_attn[2 * C + v_rows][None, :], (128, 256))).astype(np.float32)
        wpt = np.ascontiguousarray(
            (W_proj[:, ch0:ch0 + CH] / 32.0).T.reshape(2, 128, 1024)
            .transpose(1, 0, 2)).astype(NPB)

        blob8a = np.concatenate(
            [wqk_hi.reshape(128, -1), wv_hi.reshape(128, -1)], axis=1)
        blob8b = np.concatenate(
            [wqk_lo.reshape(128, -1), wv_lo.reshape(128, -1),
             S8, S8z], axis=1)
        blob16 = np.concatenate(
            [wpt.reshape(128, -1), S16, eye], axis=1)
        blob32 = np.concatenate(
            [bq2, bk2, bvb], axis=1).astype(np.float32)
        in_maps.append({
            "xhi": xhis[b], "xlo": xlos[b],
            "blob8a": np.ascontiguousarray(blob8a),
            "blob8b": np.ascontiguousarray(blob8b),
            "blob16": np.ascontiguousarray(blob16),
            "blob32": np.ascontiguousarray(blob32),
        })
    return in_maps


def kernel(x, W_attn, b_attn, W_proj, b_proj):
    x = np.asarray(x, dtype=np.float32)
    W_attn = np.asarray(W_attn, dtype=np.float32)
    b_attn = np.asarray(b_attn, dtype=np.float32)
    W_proj = np.asarray(W_proj, dtype=np.float32)
    b_proj = np.asarray(b_proj, dtype=np.float32)

    nc = _get_compiled()
    in_maps = _host_prep(x, W_attn, b_attn, W_proj, b_proj)
    res = run_bass_kernel_spmd(nc, in_maps, core_ids=list(range(N_CORES)))

    out = np.empty((B, T, C), dtype=np.float32)
    for b in range(B):
        acc = res.results[4 * b]["out_partial"].astype(np.float32)
        for g in range(1, 4):
            acc += res.results[4 * b + g]["out_partial"].astype(np.float32)
        out[b] = acc + b_proj
    return out


# revision 32
# speedup vs baseline: 1.0612x; 1.0553x over previous
"""Causal self-attention on 8 NeuronCores (Bass/Tile, fp8 DoubleRow).

Sharding: tensor-parallel over heads x data-parallel over batch.
  core c -> batch b = c//4, heads 4g..4g+3 where g = c%4.

Precision plan (validated in numpy sim, max-rel ~3.5e-3 vs 2e-2 gate):
  - tokens 0..511 ("stripe 0", small softmax sums -> no error averaging):
    3-product compensated fp8-DR QKV, bf16 scores/p/v attention.
  - tokens 512..2047: single-product fp8-DR QKV (x_hi*W_hi), fp8-DR
    scores (d=64 as 2x32 DoubleRow slots), fp8 p via ACT exp or DVE
    Schraudolph bit-trick (affine -> int8 -> e4m3 bits), fp8-DR pv with
    p stationary / v-augmented moving (65 rows per 256-token kblock pair).
  - scales folded host-side: W_qk*64 (k also /sqrt(64)), W_v*32,
    exp(psum/4096), W_proj/32; biases folded to match.
  - pv-B orientation puts the softmax denominator per-partition: one
    reciprocal [128,4,1] + one broadcast-multiply per query block.
  - yT via PE transpose (bf16), output projection bf16, bf16 partials
    DMA'd out; host sums 4 partials per batch in fp32 + b_proj.
"""

import os
import sys

for _p in ("/opt/trn_rl_repo", "/opt/pypackages"):
    if os.path.isdir(_p) and _p not in sys.path:
        sys.path.append(_p)

import numpy as np
import ml_dtypes

import concourse.bass as bass
import concourse.tile as tile
import concourse.mybir as mybir
from concourse import bacc
from concourse.bass_utils import run_bass_kernel_spmd

B, T, C = 2, 2048, 1024
H = 16            # total heads
D = 64            # head dim
HPC = 4           # heads per core
CH = HPC * D      # 256 channels per core
N_CORES = 8
NT = 4            # 512-token stripes

f32 = mybir.dt.float32
bf16 = mybir.dt.bfloat16
f8 = mybir.dt.float8e4
i8 = mybir.dt.int8
NP8 = ml_dtypes.float8_e4m3
NPB = ml_dtypes.bfloat16
ts = bass.ts
ds = bass.ds
DR = mybir.MatmulPerfMode.DoubleRow
Exp = mybir.ActivationFunctionType.Exp
MULT = mybir.AluOpType.mult
ADD = mybir.AluOpType.add

LOG2E = 1.4426950408889634
SCHR_C = 55.54            # e4m3 Schraudolph constant (DVE rounds to nearest)
EXP_SCALE = 1.0 / 4096.0  # q,k carry 64x each
DVE_FRAC = 0.40           # tail fraction of each stripe's exps on DVE

_COMPILED = None


def _build():
    nc = bacc.Bacc("TRN2", target_bir_lowering=False, debug=False,
                   num_devices=N_CORES)

    # input blobs: fp8 hi-weights (needed first), fp8 rest, bf16, f32
    N8A = 4096 + 2048                                # wqk wv
    N8B = 4096 + 2048 + 128 + 256                    # wqkl wvl S8 S8z
    N16 = 2048 + 128 + 128                           # wpt S16 eye
    N32 = 2 + 2 + 256                                # bq2 bk2 bvb
    blob8a_d = nc.dram_tensor("blob8a", [128, N8A], f8,
                              kind="ExternalInput").ap()
    blob8b_d = nc.dram_tensor("blob8b", [128, N8B], f8,
                              kind="ExternalInput").ap()
    blob16_d = nc.dram_tensor("blob16", [128, N16], bf16,
                              kind="ExternalInput").ap()
    blob32_d = nc.dram_tensor("blob32", [128, N32], f32,
                              kind="ExternalInput").ap()
    xhi_d = nc.dram_tensor("xhi", [128, 4, 2, T], f8, kind="ExternalInput").ap()
    xlo_d = nc.dram_tensor("xlo", [128, 4, 2, 512], f8, kind="ExternalInput").ap()
    out_d = nc.dram_tensor("out_partial", [T, C], bf16, kind="ExternalOutput").ap()

    with tile.TileContext(nc) as tc:
        with tc.tile_pool(name="consts", bufs=1) as consts, \
             tc.tile_pool(name="act", bufs=1) as act, \
             tc.tile_pool(name="xp", bufs=2) as xp, \
             tc.tile_pool(name="pp", bufs=34) as pp, \
             tc.tile_pool(name="p16", bufs=5) as p16p, \
             tc.tile_pool(name="ysb", bufs=3) as ysbp, \
             tc.tile_pool(name="rp", bufs=4) as rp, \
             tc.tile_pool(name="op", bufs=4) as op, \
             tc.tile_pool(name="ps_v", bufs=2, space="PSUM") as ps_v, \
             tc.tile_pool(name="ps_s", bufs=2, space="PSUM") as ps_s, \
             tc.tile_pool(name="ps_y", bufs=2, space="PSUM") as ps_y:

            # ---- constants: staged blob DMAs (hi-weights + stripe-0 x
            #      first so qkv(0) starts ASAP) ----
            blob8a = consts.tile([128, N8A], f8)
            blob8b = consts.tile([128, N8B], f8)
            blob16 = consts.tile([128, N16], bf16)
            blob32 = consts.tile([128, N32], f32)
            xhi_sb = consts.tile([128, 4, 2, T], f8)
            xlo_sb = consts.tile([128, 4, 2, 512], f8)
            nc.sync.dma_start(blob8a[:], blob8a_d)
            nc.sync.dma_start(xhi_sb[:, :, :, 0:512], xhi_d[:, :, :, 0:512])
            nc.sync.dma_start(blob8b[:], blob8b_d)
            nc.sync.dma_start(xlo_sb[:], xlo_d)
            nc.sync.dma_start(blob32[:], blob32_d)
            nc.sync.dma_start(xhi_sb[:, :, :, 512:T], xhi_d[:, :, :, 512:T])
            nc.sync.dma_start(blob16[:], blob16_d)

            wqk_sb = blob8a[:, 0:4096].rearrange("p (c s f) -> p c s f",
                                                 c=4, s=2)
            wv_sb = blob8a[:, 4096:6144].rearrange("p (c s f) -> p c s f",
                                                   c=4, s=2)
            wqkl_sb = blob8b[:, 0:4096].rearrange("p (c s f) -> p c s f",
                                                  c=4, s=2)
            wvl_sb = blob8b[:, 4096:6144].rearrange("p (c s f) -> p c s f",
                                                    c=4, s=2)
            s8_sb = blob8b[:, 6144:6272]
            s8z_sb = blob8b[:, 6272:6528]
            wpt_sb = blob16[:, 0:2048].rearrange("p (s o) -> p s o", s=2)
            s16_sb = blob16[:, 2048:2176]
            eye_sb = blob16[:, 2176:2304]
            bq2_sb = blob32[:, 0:2].rearrange("p (s o) -> p s o", o=1)
            bk2_sb = blob32[:, 2:4].rearrange("p (s o) -> p s o", o=1)
            bvb_sb = blob32[:, 4:260].rearrange("p (o h d) -> p o h d",
                                                o=1, h=HPC)

            # ---- persistent activations ----
            # per head-pair tiles (SBUF AP base partition must be 0/32/64,
            # so head 3 cannot live at partitions 96..127 of a 128-tile)
            q8s = [act.tile([64, 2, T], f8, name=f"q8_{i}") for i in range(2)]
            k8s = [act.tile([64, 2, T], f8, name=f"k8_{i}") for i in range(2)]
            qb16s = [act.tile([64, 2, 512], bf16, name=f"qb_{i}")
                     for i in range(2)]
            kb16s = [act.tile([64, 2, 512], bf16, name=f"kb_{i}")
                     for i in range(2)]
            vaug8 = act.tile([128, 16, HPC, D + 1], f8)   # [tok,blk,h,d|1] 32v
            vaugb = act.tile([128, 4, HPC, D + 1], bf16)  # stripe-0 blocks
            yT = act.tile([128, 2, T], bf16)      # [ch-in-slab, slab, t] 32y

            nc.vector.memset(vaug8[:, :, :, D:D + 1], 1.0)
            nc.vector.memset(vaugb[:, :, :, D:D + 1], 1.0)

            def qkv_parts(ti):
                """QKV for stripe ti as 4 independently-emittable parts so
                they can interleave into the previous stripe's attention."""
                xhi = xhi_sb[:, :, :, ts(ti, 512)]
                prods = [(xhi, wqk_sb, wv_sb)]
                if ti == 0:
                    prods += [(xlo_sb, wqk_sb, wv_sb), (xhi, wqkl_sb, wvl_sb)]
                n = len(prods) * 4

                def qk_part(name, f0, bias, d8, d16):
                    def run():
                        ps = ps_s.tile([128, 2, 512], f32, tag="sc",
                                       name=f"ps_{name}{ti}")
                        for a in range(2):
                            j = 0
                            for xa, wa, _ in prods:
                                for ci in range(4):
                                    nc.tensor.matmul(
                                        ps[:, a, :],
                                        wa[:, ci, :, ds(f0 + 128 * a, 128)],
                                        xa[:, ci, :, :],
                                        start=(j == 0), stop=(j == n - 1),
                                        perf_mode=DR)
                                    j += 1
                        for hh in range(2):
                            psl = ps[ds(64 * hh, 64), :, :]
                            bl = bias[ds(64 * hh, 64), :, :].to_broadcast(
                                [64, 2, 512])
                            if ti == 0:
                                nc.vector.tensor_tensor(
                                    out=d16[hh][:], in0=psl, in1=bl, op=ADD)
                                if name == "k":
                                    nc.vector.tensor_tensor(
                                        out=d8[hh][:, :, 0:512], in0=psl,
                                        in1=bl, op=ADD)
                            else:
                                nc.vector.tensor_tensor(
                                    out=d8[hh][:, :, ts(ti, 512)], in0=psl,
                                    in1=bl, op=ADD)
                    return run

                def v_part(half):
                    def run():
                        pv = ps_v.tile([128, 2, 256], f32, tag="vps",
                                       name=f"vps{ti}_{half}")
                        for tj2 in range(2):
                            tj = 2 * half + tj2
                            j = 0
                            for xa, _, wva in prods:
                                for ci in range(4):
                                    nc.tensor.matmul(
                                        pv[:, tj2, :],
                                        xa[:, ci, :, ds(128 * tj, 128)],
                                        wva[:, ci, :, :],
                                        start=(j == 0), stop=(j == n - 1),
                                        perf_mode=DR)
                                    j += 1
                        blk = 4 * ti + 2 * half
                        pv4 = pv[:].rearrange("p b (h d) -> p b h d", h=HPC)
                        bvb4 = bvb_sb.to_broadcast([128, 2, HPC, D])
                        nc.vector.tensor_tensor(
                            out=vaug8[:, ds(blk, 2), :, 0:D],
                            in0=pv4, in1=bvb4, op=ADD)
                        if ti == 0:
                            nc.vector.tensor_tensor(
                                out=vaugb[:, ds(blk, 2), :, 0:D],
                                in0=pv4, in1=bvb4, op=ADD)
                    return run

                return [qk_part("q", 0, bq2_sb, q8s, qb16s),
                        qk_part("k", 256, bk2_sb, k8s, kb16s),
                        v_part(0), v_part(1)]

            def attn0(extra=()):
                extra = list(extra)
                for h in range(HPC):
                    if extra:
                        extra.pop(0)()
                    hp, hh = 32 * (h % 2), h // 2
                    py = ps_y.tile([D + 1, 512], f32, tag="y")
                    pk = [None] * 4
                    psc = None
                    for ki in range(5):
                        if ki < 4:
                            q0 = 128 * ki
                            if ki % 2 == 0:
                                psc = ps_s.tile([128, 2, 512], f32, tag="sc",
                                                name=f"psc0_{h}_{ki}")
                            for sl in range(2):
                                nc.tensor.matmul(
                                    psc[:, ki % 2, q0:],
                                    kb16s[hh][hp:hp + 32, sl, ts(ki, 128)],
                                    qb16s[hh][hp:hp + 32, sl, q0:],
                                    start=(sl == 0), stop=(sl == 1))
                            p = p16p.tile([128, 512], bf16, tag="p16",
                                          name=f"p16_{h}_{ki}")
                            nc.scalar.activation(p[:, q0:], psc[:, ki % 2, q0:],
                                                 Exp, scale=EXP_SCALE)
                            nc.gpsimd.tensor_tensor(
                                out=p[:, q0:q0 + 128], in0=p[:, q0:q0 + 128],
                                in1=s16_sb, op=MULT)
                            pk[ki] = p
                        if ki >= 1:
                            kj = ki - 1
                            qj = 128 * kj
                            nc.tensor.matmul(
                                py[:, qj:], vaugb[:, kj, h, :], pk[kj][:, qj:],
                                start=(kj == 0), stop=(kj == 3))
                    rec = rp.tile([1, 512], f32, tag="rec")
                    nc.vector.reciprocal(rec[:], py[D:D + 1, :])
                    bc = rp.tile([D, 512], f32, tag="bc")
                    nc.gpsimd.partition_broadcast(bc[:], rec[:], channels=D)
                    nc.vector.tensor_tensor(
                        out=yT[ds(64 * (h % 2), D), h // 2, 0:512],
                        in0=py[0:D, :], in1=bc[:], op=MULT)

            def attn(qi, fuse_oproj=False, extra=()):
                extra = list(extra)
                npair = 2 * qi + 2

                def mmax(qb):
                    return (4 * qi + qb) // 2

                ph = [[None] * npair for _ in range(HPC)]
                pybs = {}
                ot = op.tile([128, 4, 1024], bf16, tag="ot", name="ot3") if fuse_oproj \
                    else None

                def scores_exp(h, m):
                    hp, hh = 32 * (h % 2), h // 2
                    diag = m >= 2 * qi
                    q0 = 256 if m == 2 * qi + 1 else 0
                    psc = ps_s.tile([128, 2, 512], f32, tag="sc")
                    for i in range(2):
                        ki = 2 * m + i
                        nc.tensor.matmul(
                            psc[:, i, q0:],
                            k8s[hh][hp:hp + 32, :, ts(ki, 128)],
                            q8s[hh][hp:hp + 32, :,
                                    ds(512 * qi + q0, 512 - q0)],
                            start=True, stop=True, perf_mode=DR)
                    p = pp.tile([128, 2, 512], f8, tag="p")
                    # interleave exp units across ACT and DVE so both run
                    # concurrently (2-deep score ring = 2 exps in flight)
                    use_dve = ((h * npair + m) % 3 == 2) and q0 == 0
                    if use_dve:
                        nc.vector.tensor_scalar(
                            out=p[:].bitcast(i8), in0=psc[:],
                            scalar1=8.0 * LOG2E * EXP_SCALE,
                            scalar2=SCHR_C, op0=MULT, op1=ADD)
                    else:
                        nc.scalar.activation(
                            p[:, :, q0:], psc[:, :, q0:],
                            Exp, scale=EXP_SCALE)
                    if diag:
                        nc.gpsimd.tensor_tensor(
                            out=p[:, 0, q0:q0 + 128],
                            in0=p[:, 0, q0:q0 + 128], in1=s8_sb, op=MULT)
                        nc.gpsimd.tensor_tensor(
                            out=p[:, 1, q0:q0 + 256],
                            in0=p[:, 1, q0:q0 + 256], in1=s8z_sb, op=MULT)
                    ph[h][m] = p

                def pvb_step(qb, h, m):
                    # one open accumulation group per bank at a time:
                    # h-major order keeps groups sequential
                    nc.tensor.matmul(
                        pybs[qb][:, h, :],
                        ph[h][m][:, :, ts(qb, 128)],
                        vaug8[:, ds(2 * m, 2), h, :],
                        start=(m == 0), stop=(m == mmax(qb)),
                        perf_mode=DR)

                def finish(qb):
                    pyb = pybs[qb]
                    rec = rp.tile([128, HPC, 1], f32, tag="recb")
                    nc.vector.reciprocal(rec[:], pyb[:, :, D:D + 1])
                    y_sb = ysbp.tile([128, HPC, D], bf16, tag="ysb")
                    nc.vector.tensor_tensor(
                        out=y_sb[:], in0=pyb[:, :, 0:D],
                        in1=rec[:].to_broadcast([128, HPC, D]), op=MULT)
                    tps = ps_y.tile([128, 2, 128], bf16, tag="y")
                    for s in range(2):
                        nc.tensor.transpose(
                            tps[:, s, :],
                            y_sb[:, ds(2 * s, 2), :].rearrange(
                                "p h d -> p (h d)"),
                            eye_sb)
                    nc.vector.tensor_copy(
                        yT[:, :, ds(512 * qi + 128 * qb, 128)], tps[:])
                    if fuse_oproj:
                        tg = 4 * qi + qb
                        for oi in range(2):
                            po = ps_s.tile([128, 512], f32, tag="sc",
                                           name=f"pof{tg}_{oi}")
                            for s in range(2):
                                nc.tensor.matmul(
                                    po[:], yT[:, s, ts(tg, 128)],
                                    wpt_sb[:, s, ts(oi, 512)],
                                    start=(s == 0), stop=(s == 1))
                            if (qb + oi) % 2 == 0:
                                nc.scalar.copy(ot[:, qb, ts(oi, 512)], po[:])
                            else:
                                nc.vector.tensor_copy(
                                    ot[:, qb, ts(oi, 512)], po[:])

                # qb 0,1 chains interleave with scores/exp at pair
                # granularity; h-major so each psum bank has exactly one
                # open accumulation group at any time
                pybs[0] = ps_y.tile([128, HPC, D + 1], f32, tag="y", name="pyb0")
                pybs[1] = ps_y.tile([128, HPC, D + 1], f32, tag="y", name="pyb1")
                LAG = 2   # pv consumes exp output LAG pairs behind the
                          # scores so in-order PE never head-of-line blocks
                for h in range(HPC):
                    if extra:
                        extra.pop(0)()
                    for m in range(npair + LAG):
                        if m < npair:
                            scores_exp(h, m)
                        mm = m - LAG
                        if mm >= 0:
                            for qb in (0, 1):
                                if mm <= mmax(qb):
                                    pvb_step(qb, h, mm)
                finish(0)
                finish(1)
                while extra:
                    extra.pop(0)()
                # qb 2,3 trail (overlap with next stripe's qkv/scores)
                pybs[2] = ps_y.tile([128, HPC, D + 1], f32, tag="y", name="pyb2")
                pybs[3] = ps_y.tile([128, HPC, D + 1], f32, tag="y", name="pyb3")
                for h in range(HPC):
                    for m in range(npair):
                        for qb in (2, 3):
                            if m <= mmax(qb):
                                pvb_step(qb, h, m)
                finish(2)
                finish(3)
                if fuse_oproj:
                    si = qi
                    nc.sync.dma_start(
                        out_d[ds(512 * si, 512), :].rearrange(
                            "(g p) c -> p g c", g=4), ot[:])

            def oproj_parts(si):
                ot = op.tile([128, 4, 1024], bf16, tag="ot", name=f"ot{si}")

                def tg_part(g):
                    def run():
                        tg = 4 * si + g
                        for oi in range(2):
                            po = ps_s.tile([128, 512], f32, tag="sc",
                                           name=f"po{tg}_{oi}")
                            for s in range(2):
                                nc.tensor.matmul(
                                    po[:], yT[:, s, ts(tg, 128)],
                                    wpt_sb[:, s, ts(oi, 512)],
                                    start=(s == 0), stop=(s == 1))
                            if (g + oi) % 2 == 0:
                                nc.scalar.copy(ot[:, g, ts(oi, 512)], po[:])
                            else:
                                nc.vector.tensor_copy(
                                    ot[:, g, ts(oi, 512)], po[:])
                        if g == 3:
                            nc.sync.dma_start(
                                out_d[ds(512 * si, 512), :].rearrange(
                                    "(g p) c -> p g c", g=4), ot[:])
                    return run
                return [tg_part(g) for g in range(4)]

            for part in qkv_parts(0):
                part()
            attn0(extra=qkv_parts(1))
            attn(1, extra=qkv_parts(2) + oproj_parts(0))
            attn(2, extra=qkv_parts(3) + oproj_parts(1))
            attn(3, fuse_oproj=True, extra=oproj_parts(2))

    nc.compile()
    return nc


def _get_compiled():
    global _COMPILED
    if _COMPILED is None:
        _COMPILED = _build()
    return _COMPILED


def _split8(a):
    hi = a.astype(NP8)
    lo = (a - hi.astype(np.float32)).astype(NP8)
    return hi, lo


def _host_prep(x, W_attn, b_attn, W_proj, b_proj):
    scale = np.float32(1.0 / np.sqrt(D))
    dd = np.arange(128)
    S8np = (np.arange(128)[None, :] >= dd[:, None])
    S8 = S8np.astype(NP8)
    S8z = np.concatenate(
        [np.zeros((128, 128), NP8), S8], axis=1)
    S16 = S8np.astype(NPB)
    eye = np.eye(128, dtype=NPB)

    # x in DR layout [p, ci, slot, t] per batch
    xhis, xlos = [], []
    for b in range(B):
        xt = np.ascontiguousarray(x[b].T)          # [C, T]
        xr = xt.reshape(4, 2, 128, T).transpose(2, 0, 1, 3)  # [p, ci, s, T]
        hi, lo = _split8(xr)
        xhis.append(np.ascontiguousarray(hi))
        xlos.append(np.ascontiguousarray(lo[:, :, :, :512]))

    in_maps = []
    for c in range(N_CORES):
        b, g = divmod(c, 4)
        ch0 = CH * g
        # output-channel orderings
        # q/k rows: f = 128*half + 32*h + dd  ->  qchan = 64*(4g+h) + 32*half + dd
        h_i = np.arange(128) // 32
        dd_i = np.arange(128) % 32
        qk_rows = np.concatenate(
            [ch0 + 64 * h_i + 32 * half + dd_i for half in range(2)])
        v_rows = ch0 + np.arange(256)              # 64*h + d order
        Wq = 64.0 * W_attn[qk_rows]                      # [256, C]
        Wk = 64.0 * scale * W_attn[C + qk_rows]
        Wv = 32.0 * W_attn[2 * C + v_rows]
        # stationary layout [p, ci, slot, f]
        def wlay(Wm):
            # Wm [F, C] -> [p, ci, s, F]
            r = Wm.T.reshape(4, 2, 128, Wm.shape[0]).transpose(2, 0, 1, 3)
            return np.ascontiguousarray(r)
        Wqk = np.concatenate([wlay(Wq), wlay(Wk)], axis=3)  # [p,ci,s,512]
        Wvl = wlay(Wv)                                       # [p,ci,s,256]
        wqk_hi, wqk_lo = _split8(Wqk)
        wv_hi, wv_lo = _split8(Wvl)

        bq2 = np.stack([64.0 * b_attn[ch0 + 64 * h_i + 32 * half + dd_i]
                        for half in range(2)], axis=1).astype(np.float32)
        bk2 = np.stack([64.0 * scale * b_attn[C + ch0 + 64 * h_i + 32 * half + dd_i]
                        for half in range(2)], axis=1).astype(np.float32)
        bvb = np.ascontiguousarray(np.broadcast_to(
            32.0 * b_attn[2 * C + v_rows][None, :], (128, 256))).astype(np.float32)
        wpt = np.ascontiguousarray(
            (W_proj[:, ch0:ch0 + CH] / 32.0).T.reshape(2, 128, 1024)
            .transpose(1, 0, 2)).astype(NPB)

        blob8a = np.concatenate(
            [wqk_hi.reshape(128, -1), wv_hi.reshape(128, -1)], axis=1)
        blob8b = np.concatenate(
            [wqk_lo.reshape(128, -1), wv_lo.reshape(128, -1),
             S8, S8z], axis=1)
        blob16 = np.concatenate(
            [wpt.reshape(128, -1), S16, eye], axis=1)
        blob32 = np.concatenate(
            [bq2, bk2, bvb], axis=1).astype(np.float32)
        in_maps.append({
            "xhi": xhis[b], "xlo": xlos[b],
            "blob8a": np.ascontiguousarray(blob8a),
            "blob8b": np.ascontiguousarray(blob8b),
            "blob16": np.ascontiguousarray(blob16),
            "blob32": np.ascontiguousarray(blob32),
        })
    return in_maps


def kernel(x, W_attn, b_attn, W_proj, b_proj):
    x = np.asarray(x, dtype=np.float32)
    W_attn = np.asarray(W_attn, dtype=np.float32)
    b_attn = np.asarray(b_attn, dtype=np.float32)
    W_proj = np.asarray(W_proj, dtype=np.float32)
    b_proj = np.asarray(b_proj, dtype=np.float32)

    nc = _get_compiled()
    in_maps = _host_prep(x, W_attn, b_attn, W_proj, b_proj)
    res = run_bass_kernel_spmd(nc, in_maps, core_ids=list(range(N_CORES)))

    out = np.empty((B, T, C), dtype=np.float32)
    for b in range(B):
        acc = res.results[4 * b]["out_partial"].astype(np.float32)
        for g in range(1, 4):
            acc += res.results[4 * b + g]["out_partial"].astype(np.float32)
        out[b] = acc + b_proj
    return out


# revision 34
# speedup vs baseline: 1.0768x; 1.0147x over previous
"""Causal self-attention on 8 NeuronCores (Bass/Tile, fp8 DoubleRow).

Sharding: tensor-parallel over heads x data-parallel over batch.
  core c -> batch b = c//4, heads 4g..4g+3 where g = c%4.

Precision plan (validated in numpy sim, max-rel ~3.5e-3 vs 2e-2 gate):
  - tokens 0..511 ("stripe 0", small softmax sums -> no error averaging):
    3-product compensated fp8-DR QKV, bf16 scores/p/v attention.
  - tokens 512..2047: single-product fp8-DR QKV (x_hi*W_hi), fp8-DR
    scores (d=64 as 2x32 DoubleRow slots), fp8 p via ACT exp or DVE
    Schraudolph bit-trick (affine -> int8 -> e4m3 bits), fp8-DR pv with
    p stationary / v-augmented moving (65 rows per 256-token kblock pair).
  - scales folded host-side: W_qk*64 (k also /sqrt(64)), W_v*32,
    exp(psum/4096), W_proj/32; biases folded to match.
  - pv-B orientation puts the softmax denominator per-partition: one
    reciprocal [128,4,1] + one broadcast-multiply per query block.
  - yT via PE transpose (bf16), output projection bf16, bf16 partials
    DMA'd out; host sums 4 partials per batch in fp32 + b_proj.
"""

import os
import sys

for _p in ("/opt/trn_rl_repo", "/opt/pypackages"):
    if os.path.isdir(_p) and _p not in sys.path:
        sys.path.append(_p)

import numpy as np
import ml_dtypes

import concourse.bass as bass
import concourse.tile as tile
import concourse.mybir as mybir
from concourse import bacc
from concourse.bass_utils import run_bass_kernel_spmd

B, T, C = 2, 2048, 1024
H = 16            # total heads
D = 64            # head dim
HPC = 4           # heads per core
CH = HPC * D      # 256 channels per core
N_CORES = 8
NT = 4            # 512-token stripes

f32 = mybir.dt.float32
bf16 = mybir.dt.bfloat16
f8 = mybir.dt.float8e4
i8 = mybir.dt.int8
NP8 = ml_dtypes.float8_e4m3
NPB = ml_dtypes.bfloat16
ts = bass.ts
ds = bass.ds
DR = mybir.MatmulPerfMode.DoubleRow
Exp = mybir.ActivationFunctionType.Exp
MULT = mybir.AluOpType.mult
ADD = mybir.AluOpType.add

LOG2E = 1.4426950408889634
SCHR_C = 55.54            # e4m3 Schraudolph constant (DVE rounds to nearest)
EXP_SCALE = 1.0 / 4096.0  # q,k carry 64x each
DVE_FRAC = 0.40           # tail fraction of each stripe's exps on DVE

_COMPILED = None


def _build():
    nc = bacc.Bacc("TRN2", target_bir_lowering=False, debug=False,
                   num_devices=N_CORES)

    # input blobs: fp8 hi-weights (needed first), fp8 rest, bf16, f32
    N8A = 4096 + 2048                                # wqk wv
    N8B = 4096 + 2048 + 128 + 256                    # wqkl wvl S8 S8z
    N16 = 2048 + 128 + 128                           # wpt S16 eye
    N32 = 2 + 2 + 256                                # bq2 bk2 bvb
    blob8a_d = nc.dram_tensor("blob8a", [128, N8A], f8,
                              kind="ExternalInput").ap()
    blob8b_d = nc.dram_tensor("blob8b", [128, N8B], f8,
                              kind="ExternalInput").ap()
    blob16_d = nc.dram_tensor("blob16", [128, N16], bf16,
                              kind="ExternalInput").ap()
    blob32_d = nc.dram_tensor("blob32", [128, N32], f32,
                              kind="ExternalInput").ap()
    xhi_d = nc.dram_tensor("xhi", [128, 4, 2, T], f8, kind="ExternalInput").ap()
    xlo_d = nc.dram_tensor("xlo", [128, 4, 2, 512], f8, kind="ExternalInput").ap()
    out_d = nc.dram_tensor("out_partial", [T, C], bf16, kind="ExternalOutput").ap()

    with tile.TileContext(nc) as tc:
        with tc.tile_pool(name="consts", bufs=1) as consts, \
             tc.tile_pool(name="act", bufs=1) as act, \
             tc.tile_pool(name="xp", bufs=2) as xp, \
             tc.tile_pool(name="pp", bufs=34) as pp, \
             tc.tile_pool(name="p16", bufs=5) as p16p, \
             tc.tile_pool(name="ysb", bufs=3) as ysbp, \
             tc.tile_pool(name="rp", bufs=4) as rp, \
             tc.tile_pool(name="op", bufs=4) as op, \
             tc.tile_pool(name="ps_v", bufs=2, space="PSUM") as ps_v, \
             tc.tile_pool(name="ps_s", bufs=2, space="PSUM") as ps_s, \
             tc.tile_pool(name="ps_y", bufs=2, space="PSUM") as ps_y:

            # ---- constants: staged blob DMAs (hi-weights + stripe-0 x
            #      first so qkv(0) starts ASAP) ----
            blob8a = consts.tile([128, N8A], f8)
            blob8b = consts.tile([128, N8B], f8)
            blob16 = consts.tile([128, N16], bf16)
            blob32 = consts.tile([128, N32], f32)
            xhi_sb = consts.tile([128, 4, 2, T], f8)
            xlo_sb = consts.tile([128, 4, 2, 512], f8)
            nc.sync.dma_start(blob8a[:], blob8a_d)
            nc.sync.dma_start(xhi_sb[:, :, :, 0:512], xhi_d[:, :, :, 0:512])
            nc.sync.dma_start(blob8b[:], blob8b_d)
            nc.sync.dma_start(xlo_sb[:], xlo_d)
            nc.sync.dma_start(blob32[:], blob32_d)
            nc.sync.dma_start(xhi_sb[:, :, :, 512:T], xhi_d[:, :, :, 512:T])
            nc.sync.dma_start(blob16[:], blob16_d)

            wqk_sb = blob8a[:, 0:4096].rearrange("p (c s f) -> p c s f",
                                                 c=4, s=2)
            wv_sb = blob8a[:, 4096:6144].rearrange("p (c s f) -> p c s f",
                                                   c=4, s=2)
            wqkl_sb = blob8b[:, 0:4096].rearrange("p (c s f) -> p c s f",
                                                  c=4, s=2)
            wvl_sb = blob8b[:, 4096:6144].rearrange("p (c s f) -> p c s f",
                                                    c=4, s=2)
            s8_sb = blob8b[:, 6144:6272]
            s8z_sb = blob8b[:, 6272:6528]
            wpt_sb = blob16[:, 0:2048].rearrange("p (s o) -> p s o", s=2)
            s16_sb = blob16[:, 2048:2176]
            eye_sb = blob16[:, 2176:2304]
            bq2_sb = blob32[:, 0:2].rearrange("p (s o) -> p s o", o=1)
            bk2_sb = blob32[:, 2:4].rearrange("p (s o) -> p s o", o=1)
            bvb_sb = blob32[:, 4:260].rearrange("p (o h d) -> p o h d",
                                                o=1, h=HPC)

            # ---- persistent activations ----
            # per head-pair tiles (SBUF AP base partition must be 0/32/64,
            # so head 3 cannot live at partitions 96..127 of a 128-tile)
            q8s = [act.tile([64, 2, T], f8, name=f"q8_{i}") for i in range(2)]
            k8s = [act.tile([64, 2, T], f8, name=f"k8_{i}") for i in range(2)]
            qb16s = [act.tile([64, 2, 512], bf16, name=f"qb_{i}")
                     for i in range(2)]
            kb16s = [act.tile([64, 2, 512], bf16, name=f"kb_{i}")
                     for i in range(2)]
            vaug8 = act.tile([128, 16, HPC, D + 1], f8)   # [tok,blk,h,d|1] 32v
            vaugb = act.tile([128, 4, HPC, D + 1], bf16)  # stripe-0 blocks
            yT = act.tile([128, 2, T], bf16)      # [ch-in-slab, slab, t] 32y

            nc.vector.memset(vaug8[:, :, :, D:D + 1], 1.0)
            nc.vector.memset(vaugb[:, :, :, D:D + 1], 1.0)

            def qkv_parts(ti):
                """QKV for stripe ti as 4 independently-emittable parts so
                they can interleave into the previous stripe's attention."""
                xhi = xhi_sb[:, :, :, ts(ti, 512)]
                prods = [(xhi, wqk_sb, wv_sb)]
                if ti == 0:
                    prods += [(xlo_sb, wqk_sb, wv_sb), (xhi, wqkl_sb, wvl_sb)]
                n = len(prods) * 4

                def qk_part(name, f0, bias, d8, d16):
                    def run():
                        ps = ps_s.tile([128, 2, 512], f32, tag="sc",
                                       name=f"ps_{name}{ti}")
                        for a in range(2):
                            j = 0
                            for xa, wa, _ in prods:
                                for ci in range(4):
                                    nc.tensor.matmul(
                                        ps[:, a, :],
                                        wa[:, ci, :, ds(f0 + 128 * a, 128)],
                                        xa[:, ci, :, :],
                                        start=(j == 0), stop=(j == n - 1),
                                        perf_mode=DR)
                                    j += 1
                        for hh in range(2):
                            psl = ps[ds(64 * hh, 64), :, :]
                            bl = bias[ds(64 * hh, 64), :, :].to_broadcast(
                                [64, 2, 512])
                            if ti == 0:
                                nc.vector.tensor_tensor(
                                    out=d16[hh][:], in0=psl, in1=bl, op=ADD)
                                if name == "k":
                                    nc.vector.tensor_tensor(
                                        out=d8[hh][:, :, 0:512], in0=psl,
                                        in1=bl, op=ADD)
                            else:
                                nc.vector.tensor_tensor(
                                    out=d8[hh][:, :, ts(ti, 512)], in0=psl,
                                    in1=bl, op=ADD)
                    return run

                def v_part(half):
                    def run():
                        pv = ps_v.tile([128, 2, 256], f32, tag="vps",
                                       name=f"vps{ti}_{half}")
                        for tj2 in range(2):
                            tj = 2 * half + tj2
                            j = 0
                            for xa, _, wva in prods:
                                for ci in range(4):
                                    nc.tensor.matmul(
                                        pv[:, tj2, :],
                                        xa[:, ci, :, ds(128 * tj, 128)],
                                        wva[:, ci, :, :],
                                        start=(j == 0), stop=(j == n - 1),
                                        perf_mode=DR)
                                    j += 1
                        blk = 4 * ti + 2 * half
                        pv4 = pv[:].rearrange("p b (h d) -> p b h d", h=HPC)
                        bvb4 = bvb_sb.to_broadcast([128, 2, HPC, D])
                        nc.vector.tensor_tensor(
                            out=vaug8[:, ds(blk, 2), :, 0:D],
                            in0=pv4, in1=bvb4, op=ADD)
                        if ti == 0:
                            nc.vector.tensor_tensor(
                                out=vaugb[:, ds(blk, 2), :, 0:D],
                                in0=pv4, in1=bvb4, op=ADD)
                    return run

                return [qk_part("q", 0, bq2_sb, q8s, qb16s),
                        qk_part("k", 256, bk2_sb, k8s, kb16s),
                        v_part(0), v_part(1)]

            def attn0(extra=()):
                extra = list(extra)
                for h in range(HPC):
                    if extra:
                        extra.pop(0)()
                    hp, hh = 32 * (h % 2), h // 2
                    py = ps_y.tile([D + 1, 512], f32, tag="y")
                    pk = [None] * 4
                    psc = None
                    for ki in range(5):
                        if ki < 4:
                            q0 = 128 * ki
                            if ki % 2 == 0:
                                psc = ps_s.tile([128, 2, 512], f32, tag="sc",
                                                name=f"psc0_{h}_{ki}")
                            for sl in range(2):
                                nc.tensor.matmul(
                                    psc[:, ki % 2, q0:],
                                    kb16s[hh][hp:hp + 32, sl, ts(ki, 128)],
                                    qb16s[hh][hp:hp + 32, sl, q0:],
                                    start=(sl == 0), stop=(sl == 1))
                            p = p16p.tile([128, 512], bf16, tag="p16",
                                          name=f"p16_{h}_{ki}")
                            nc.scalar.activation(p[:, q0:], psc[:, ki % 2, q0:],
                                                 Exp, scale=EXP_SCALE)
                            nc.gpsimd.tensor_tensor(
                                out=p[:, q0:q0 + 128], in0=p[:, q0:q0 + 128],
                                in1=s16_sb, op=MULT)
                            pk[ki] = p
                        if ki >= 1:
                            kj = ki - 1
                            qj = 128 * kj
                            nc.tensor.matmul(
                                py[:, qj:], vaugb[:, kj, h, :], pk[kj][:, qj:],
                                start=(kj == 0), stop=(kj == 3))
                    rec = rp.tile([1, 512], f32, tag="rec")
                    nc.vector.reciprocal(rec[:], py[D:D + 1, :])
                    bc = rp.tile([D, 512], f32, tag="bc")
                    nc.gpsimd.partition_broadcast(bc[:], rec[:], channels=D)
                    nc.vector.tensor_tensor(
                        out=yT[ds(64 * (h % 2), D), h // 2, 0:512],
                        in0=py[0:D, :], in1=bc[:], op=MULT)

            def attn(qi, fuse_oproj=False, extra=()):
                extra = list(extra)
                npair = 2 * qi + 2

                def mmax(qb):
                    return (4 * qi + qb) // 2

                ph = [[None] * npair for _ in range(HPC)]
                pybs = {}
                ot = op.tile([128, 4, 1024], bf16, tag="ot", name="ot3") if fuse_oproj \
                    else None

                def scores_exp(h, m):
                    hp, hh = 32 * (h % 2), h // 2
                    diag = m >= 2 * qi
                    q0 = 256 if m == 2 * qi + 1 else 0
                    psc = ps_s.tile([128, 2, 512], f32, tag="sc")
                    for i in range(2):
                        ki = 2 * m + i
                        nc.tensor.matmul(
                            psc[:, i, q0:],
                            k8s[hh][hp:hp + 32, :, ts(ki, 128)],
                            q8s[hh][hp:hp + 32, :,
                                    ds(512 * qi + q0, 512 - q0)],
                            start=True, stop=True, perf_mode=DR)
                    p = pp.tile([128, 2, 512], f8, tag="p")
                    # interleave exp units across ACT and DVE so both run
                    # concurrently (2-deep score ring = 2 exps in flight)
                    use_dve = (h * npair + m) % 2 == 1
                    if use_dve:
                        nc.vector.tensor_scalar(
                            out=p[:].bitcast(i8), in0=psc[:],
                            scalar1=8.0 * LOG2E * EXP_SCALE,
                            scalar2=SCHR_C, op0=MULT, op1=ADD)
                    else:
                        nc.scalar.activation(
                            p[:, :, q0:], psc[:, :, q0:],
                            Exp, scale=EXP_SCALE)
                    if diag:
                        nc.gpsimd.tensor_tensor(
                            out=p[:, 0, q0:q0 + 128],
                            in0=p[:, 0, q0:q0 + 128], in1=s8_sb, op=MULT)
                        nc.gpsimd.tensor_tensor(
                            out=p[:, 1, q0:q0 + 256],
                            in0=p[:, 1, q0:q0 + 256], in1=s8z_sb, op=MULT)
                    ph[h][m] = p

                def pvb_step(qb, h, m):
                    # one open accumulation group per bank at a time:
                    # h-major order keeps groups sequential
                    nc.tensor.matmul(
                        pybs[qb][:, h, :],
                        ph[h][m][:, :, ts(qb, 128)],
                        vaug8[:, ds(2 * m, 2), h, :],
                        start=(m == 0), stop=(m == mmax(qb)),
                        perf_mode=DR)

                def finish(qb):
                    pyb = pybs[qb]
                    rec = rp.tile([128, HPC, 1], f32, tag="recb")
                    nc.vector.reciprocal(rec[:], pyb[:, :, D:D + 1])
                    y_sb = ysbp.tile([128, HPC, D], bf16, tag="ysb")
                    nc.vector.tensor_tensor(
                        out=y_sb[:], in0=pyb[:, :, 0:D],
                        in1=rec[:].to_broadcast([128, HPC, D]), op=MULT)
                    tps = ps_y.tile([128, 2, 128], bf16, tag="y")
                    for s in range(2):
                        nc.tensor.transpose(
                            tps[:, s, :],
                            y_sb[:, ds(2 * s, 2), :].rearrange(
                                "p h d -> p (h d)"),
                            eye_sb)
                    nc.vector.tensor_copy(
                        yT[:, :, ds(512 * qi + 128 * qb, 128)], tps[:])
                    if fuse_oproj:
                        tg = 4 * qi + qb
                        for oi in range(2):
                            po = ps_s.tile([128, 512], f32, tag="sc",
                                           name=f"pof{tg}_{oi}")
                            for s in range(2):
                                nc.tensor.matmul(
                                    po[:], yT[:, s, ts(tg, 128)],
                                    wpt_sb[:, s, ts(oi, 512)],
                                    start=(s == 0), stop=(s == 1))
                            if (qb + oi) % 2 == 0:
                                nc.scalar.copy(ot[:, qb, ts(oi, 512)], po[:])
                            else:
                                nc.vector.tensor_copy(
                                    ot[:, qb, ts(oi, 512)], po[:])

                # qb 0,1 chains interleave with scores/exp at pair
                # granularity; h-major so each psum bank has exactly one
                # open accumulation group at any time
                pybs[0] = ps_y.tile([128, HPC, D + 1], f32, tag="y", name="pyb0")
                pybs[1] = ps_y.tile([128, HPC, D + 1], f32, tag="y", name="pyb1")
                LAG = 2   # pv consumes exp output LAG pairs behind the
                          # scores so in-order PE never head-of-line blocks
                for h in range(HPC):
                    if extra:
                        extra.pop(0)()
                    for m in range(npair + LAG):
                        if m < npair:
                            scores_exp(h, m)
                        mm = m - LAG
                        if mm >= 0:
                            for qb in (0, 1):
                                if mm <= mmax(qb):
                                    pvb_step(qb, h, mm)
                finish(0)
                finish(1)
                while extra:
                    extra.pop(0)()
                # qb 2,3 trail (overlap with next stripe's qkv/scores)
                pybs[2] = ps_y.tile([128, HPC, D + 1], f32, tag="y", name="pyb2")
                pybs[3] = ps_y.tile([128, HPC, D + 1], f32, tag="y", name="pyb3")
                for h in range(HPC):
                    for m in range(npair):
                        for qb in (2, 3):
                            if m <= mmax(qb):
                                pvb_step(qb, h, m)
                finish(2)
                finish(3)
                if fuse_oproj:
                    si = qi
                    nc.sync.dma_start(
                        out_d[ds(512 * si, 512), :].rearrange(
                            "(g p) c -> p g c", g=4), ot[:])

            def oproj_parts(si):
                ot = op.tile([128, 4, 1024], bf16, tag="ot", name=f"ot{si}")

                def tg_part(g):
                    def run():
                        tg = 4 * si + g
                        for oi in range(2):
                            po = ps_s.tile([128, 512], f32, tag="sc",
                                           name=f"po{tg}_{oi}")
                            for s in range(2):
                                nc.tensor.matmul(
                                    po[:], yT[:, s, ts(tg, 128)],
                                    wpt_sb[:, s, ts(oi, 512)],
                                    start=(s == 0), stop=(s == 1))
                            if (g + oi) % 2 == 0:
                                nc.scalar.copy(ot[:, g, ts(oi, 512)], po[:])
                            else:
                                nc.vector.tensor_copy(
                                    ot[:, g, ts(oi, 512)], po[:])
                        if g == 3:
                            nc.sync.dma_start(
                                out_d[ds(512 * si, 512), :].rearrange(
                                    "(g p) c -> p g c", g=4), ot[:])
                    return run
                return [tg_part(g) for g in range(4)]

            for part in qkv_parts(0):
                part()
            attn0(extra=qkv_parts(1))
            attn(1, extra=qkv_parts(2) + oproj_parts(0))
            attn(2, extra=qkv_parts(3) + oproj_parts(1))
            attn(3, fuse_oproj=True, extra=oproj_parts(2))

    nc.compile()
    return nc


def _get_compiled():
    global _COMPILED
    if _COMPILED is None:
        _COMPILED = _build()
    return _COMPILED


def _split8(a):
    hi = a.astype(NP8)
    lo = (a - hi.astype(np.float32)).astype(NP8)
    return hi, lo


def _host_prep(x, W_attn, b_attn, W_proj, b_proj):
    scale = np.float32(1.0 / np.sqrt(D))
    dd = np.arange(128)
    S8np = (np.arange(128)[None, :] >= dd[:, None])
    S8 = S8np.astype(NP8)
    S8z = np.concatenate(
        [np.zeros((128, 128), NP8), S8], axis=1)
    S16 = S8np.astype(NPB)
    eye = np.eye(128, dtype=NPB)

    # x in DR layout [p, ci, slot, t] per batch
    xhis, xlos = [], []
    for b in range(B):
        xt = np.ascontiguousarray(x[b].T)          # [C, T]
        xr = xt.reshape(4, 2, 128, T).transpose(2, 0, 1, 3)  # [p, ci, s, T]
        hi, lo = _split8(xr)
        xhis.append(np.ascontiguousarray(hi))
        xlos.append(np.ascontiguousarray(lo[:, :, :, :512]))

    in_maps = []
    for c in range(N_CORES):
        b, g = divmod(c, 4)
        ch0 = CH * g
        # output-channel orderings
        # q/k rows: f = 128*half + 32*h + dd  ->  qchan = 64*(4g+h) + 32*half + dd
        h_i = np.arange(128) // 32
        dd_i = np.arange(128) % 32
        qk_rows = np.concatenate(
            [ch0 + 64 * h_i + 32 * half + dd_i for half in range(2)])
        v_rows = ch0 + np.arange(256)              # 64*h + d order
        Wq = 64.0 * W_attn[qk_rows]                      # [256, C]
        Wk = 64.0 * scale * W_attn[C + qk_rows]
        Wv = 32.0 * W_attn[2 * C + v_rows]
        # stationary layout [p, ci, slot, f]
        def wlay(Wm):
            # Wm [F, C] -> [p, ci, s, F]
            r = Wm.T.reshape(4, 2, 128, Wm.shape[0]).transpose(2, 0, 1, 3)
            return np.ascontiguousarray(r)
        Wqk = np.concatenate([wlay(Wq), wlay(Wk)], axis=3)  # [p,ci,s,512]
        Wvl = wlay(Wv)                                       # [p,ci,s,256]
        wqk_hi, wqk_lo = _split8(Wqk)
        wv_hi, wv_lo = _split8(Wvl)

        bq2 = np.stack([64.0 * b_attn[ch0 + 64 * h_i + 32 * half + dd_i]
                        for half in range(2)], axis=1).astype(np.float32)
        bk2 = np.stack([64.0 * scale * b_attn[C + ch0 + 64 * h_i + 32 * half + dd_i]
                        for half in range(2)], axis=1).astype(np.float32)
        bvb = np.ascontiguousarray(np.broadcast_to(
            32.0 * b_attn[2 * C + v_rows][None, :], (128, 256))).astype(np.float32)
        wpt = np.ascontiguousarray(
            (W_proj[:, ch0:ch0 + CH] / 32.0).T.reshape(2, 128, 1024)
            .transpose(1, 0, 2)).astype(NPB)

        blob8a = np.concatenate(
            [wqk_hi.reshape(128, -1), wv_hi.reshape(128, -1)], axis=1)
        blob8b = np.concatenate(
            [wqk_lo.reshape(128, -1), wv_lo.reshape(128, -1),
             S8, S8z], axis=1)
        blob16 = np.concatenate(
            [wpt.reshape(128, -1), S16, eye], axis=1)
        blob32 = np.concatenate(
            [bq2, bk2, bvb], axis=1).astype(np.float32)
        in_maps.append({
            "xhi": xhis[b], "xlo": xlos[b],
            "blob8a": np.ascontiguousarray(blob8a),
            "blob8b": np.ascontiguousarray(blob8b),
            "blob16": np.ascontiguousarray(blob16),
            "blob32": np.ascontiguousarray(blob32),
        })
    return in_maps


def kernel(x, W_attn, b_attn, W_proj, b_proj):
    x = np.asarray(x, dtype=np.float32)
    W_attn = np.asarray(W_attn, dtype=np.float32)
    b_attn = np.asarray(b_attn, dtype=np.float32)
    W_proj = np.asarray(W_proj, dtype=np.float32)
    b_proj = np.asarray(b_proj, dtype=np.float32)

    nc = _get_compiled()
    in_maps = _host_prep(x, W_attn, b_attn, W_proj, b_proj)
    res = run_bass_kernel_spmd(nc, in_maps, core_ids=list(range(N_CORES)))

    out = np.empty((B, T, C), dtype=np.float32)
    for b in range(B):
        acc = res.results[4 * b]["out_partial"].astype(np.float32)
        for g in range(1, 4):
            acc += res.results[4 * b + g]["out_partial"].astype(np.float32)
        out[b] = acc + b_proj
    return out
